# revision 34
# baseline (speedup 1.0000x reference)
"""Multi-head attention (B=2, L=2048, D=1024, H=16) on 8 TRN2 NeuronCores.

Sharding: core c handles batch b=c//4 and heads 4*(c%4) .. 4*(c%4)+3
(tensor-parallel over heads x data-parallel over batch). Each core computes a
partial [L, D] output (its heads' contribution through wo); the host sums the
4 partials per batch and adds bo.

Device-side design:
  - QKV projections run in fp8-e4m3 DoubleRow matmuls (0.5 cyc/row, two
    128-row contractions per instruction). Accuracy is preserved with a
    compensated split prepared on the host: x = xh + xl, w = wh + wl (wl/xl
    are the fp8 quantization residuals), and x@w is computed as
    xh@wh + xl@wh + xh@wl (the dropped xl@wl term is ~1e-3 relative).
    Weights are pre-scaled by 32 so their uniform(-1/32,1/32) range stays in
    fp8 normal range; biases are pre-scaled to match, and the V "ones"
    column carries the same scale so softmax normalization cancels it.
  - Q^T/K^T [2*Dh, L]; S^T [k, q] blocks of [128, 512] with exp on paired
    2-bank PSUM tiles; causal handled by skipping k-blocks above the
    diagonal, shrinking diagonal tiles to their unmasked column range, and
    bf16 multiplicative mask tiles for the intra-block triangles.
  - PV runs transposed-back: O [q, dh] via lhsT = P^T slice (full 128-wide
    stationary, N=65 moving V+ones) which halves PV row count vs the
    [dh, q] orientation and makes the softmax denominator per-partition
    (one reciprocal + tensor_scalar per q-subblock, no PE broadcast).
  - O [q, dh] -> O^T via XBAR DMA transposes (SBUF->SBUF, 14ns/tile).
  - Projections/attention/output are software-pipelined per 512-row q-chunk:
    within a chunk the two head-streams of a pair interleave at pair-tile
    granularity with PV one round behind S (hiding exp latency), and a
    filler queue spreads next-chunk projections and deferred output
    projections into the PE stream's exp-wait windows. Output-projection
    units of early chunks are deferred to the exp-bound late chunks, which
    would otherwise starve the PE. The partial output is written bf16 (the
    host accumulates cores in f32).
"""

from contextlib import ExitStack

import numpy as np
import ml_dtypes

import concourse.bass as bass
import concourse.mybir as mybir
import concourse.tile as tile
from concourse import bacc
from concourse.bass_utils import run_bass_kernel_spmd

B, L, D, H = 2, 2048, 1024, 16
DH = D // H          # 64
P = 128              # partitions
NPAIR = 2            # head pairs per core (4 heads)
LQB = 512            # q chunk
NLQ = L // LQB       # 4
NKB = L // P         # 16 k blocks
KD = D // P          # 8 contraction blocks over D
N_CORES = 8
WS = 32.0            # host-side weight scale (fp8 range)
SC = (1.0 / np.sqrt(DH)) / (WS * WS)   # exp scale: undo WS^2 in scores

F32 = mybir.dt.float32
BF16 = mybir.dt.bfloat16
FP8 = mybir.dt.float8e4
AF = mybir.ActivationFunctionType
DR = mybir.MatmulPerfMode.DoubleRow
E4M3 = ml_dtypes.float8_e4m3
BF16NP = ml_dtypes.bfloat16


def build_module(iters=1, dbg=False):
    nc = bacc.Bacc("TRN2", target_bir_lowering=False, debug=False,
                   num_devices=N_CORES)
    if dbg:
        dbg_qt = nc.dram_tensor("dbg_qt", [P, L], F32, kind="ExternalOutput").ap()
        dbg_kt = nc.dram_tensor("dbg_kt", [P, L], F32, kind="ExternalOutput").ap()
        dbg_vx = nc.dram_tensor("dbg_vx", [P, NKB, 2, DH + 1], F32,
                                kind="ExternalOutput").ap()
        dbg_osl = nc.dram_tensor("dbg_osl", [P, NKB, P], F32,
                                 kind="ExternalOutput").ap()
        dbg_otT = nc.dram_tensor("dbg_otT", [P, NKB, P], F32,
                                 kind="ExternalOutput").ap()

    xh_d = nc.dram_tensor("xh", [P, KD, L], FP8, kind="ExternalInput").ap()
    xl_d = nc.dram_tensor("xl", [P, KD, L], FP8, kind="ExternalInput").ap()
    # [hl, p, g2, slot, pair, m]
    wq_d = nc.dram_tensor("wq8", [2, P, 4, 2, 2, P], FP8, kind="ExternalInput").ap()
    wk_d = nc.dram_tensor("wk8", [2, P, 4, 2, 2, P], FP8, kind="ExternalInput").ap()
    # [hl, p, g2, slot, ch]
    wv_d = nc.dram_tensor("wv8", [2, P, 4, 2, 2 * P], FP8, kind="ExternalInput").ap()
    wo_d = nc.dram_tensor("wo", [2, P, D], BF16, kind="ExternalInput").ap()
    bq_d = nc.dram_tensor("bq", [P, 2], F32, kind="ExternalInput").ap()
    bk_d = nc.dram_tensor("bk", [P, 2], F32, kind="ExternalInput").ap()
    bv_d = nc.dram_tensor("bv", [2 * P], F32, kind="ExternalInput").ap()
    mask_d = nc.dram_tensor("mask", [P, 4, LQB], BF16, kind="ExternalInput").ap()
    out = nc.dram_tensor("out", [L, D], BF16, kind="ExternalOutput").ap()

    with tile.TileContext(nc) as tc, ExitStack() as ctx:
        ctx.enter_context(
            nc.allow_low_precision(reason="fp8/bf16 matmul data path"))
        consts = ctx.enter_context(tc.tile_pool(name="consts", bufs=1))
        pers = ctx.enter_context(tc.tile_pool(name="pers", bufs=1))
        work = ctx.enter_context(tc.tile_pool(name="work", bufs=1))
        ps = ctx.enter_context(tc.tile_pool(name="ps", bufs=1, space="PSUM"))

        # ---- const tiles ---------------------------------------------------
        xh_sb = consts.tile([P, KD, L], FP8, tag="xh")
        xl_sb = consts.tile([P, KD, L], FP8, tag="xl")
        wq_sb = consts.tile([P, 2, 4, 2, 2, P], FP8, tag="wq")
        wk_sb = consts.tile([P, 2, 4, 2, 2, P], FP8, tag="wk")
        wv_sb = consts.tile([P, 2, 4, 2, 2 * P], FP8, tag="wv")
        wo_sb = consts.tile([P, 2, D], BF16, tag="wo")
        bq_sb = consts.tile([P, 2], F32, tag="bq")
        bk_sb = consts.tile([P, 2], F32, tag="bk")
        bv_bc = consts.tile([P, 2 * P], F32, tag="bv")
        mask_sb = consts.tile([P, 4, LQB], BF16, tag="mask")

        # DMA order matters: weights for chunk-0 QK first, then x chunk
        # slabs interleaved with the tensors each chunk unlocks.
        def load_x_chunk(ci, hl):
            sl = slice(ci * LQB, (ci + 1) * LQB)
            src = (xh_d, xl_d)[hl]
            dst = (xh_sb, xl_sb)[hl]
            nc.sync.dma_start(out=dst[:, :, sl], in_=src[:, :, sl])

        nc.sync.dma_start(out=wq_sb[:, 0], in_=wq_d[0])
        load_x_chunk(0, 0)
        nc.sync.dma_start(out=wk_sb[:, 0], in_=wk_d[0])
        nc.sync.dma_start(out=bq_sb[:], in_=bq_d)
        nc.sync.dma_start(out=bk_sb[:], in_=bk_d)
        load_x_chunk(0, 1)
        nc.sync.dma_start(out=wq_sb[:, 1], in_=wq_d[1])
        nc.sync.dma_start(out=wk_sb[:, 1], in_=wk_d[1])
        nc.sync.dma_start(out=wv_sb[:, 0], in_=wv_d[0])
        nc.sync.dma_start(out=wv_sb[:, 1], in_=wv_d[1])
        bv_b = bass.AP(tensor=bv_d.tensor, offset=bv_d.offset,
                       ap=[[0, P]] + list(bv_d.ap))
        nc.gpsimd.dma_start(out=bv_bc[:], in_=bv_b)
        nc.sync.dma_start(out=mask_sb[:], in_=mask_d)
        load_x_chunk(1, 0)
        load_x_chunk(1, 1)
        for pair in range(2):
            nc.sync.dma_start(out=wo_sb[:, pair], in_=wo_d[pair])
        for ci in range(2, NLQ):
            load_x_chunk(ci, 0)
            load_x_chunk(ci, 1)

        # ---- persistent work tiles ----------------------------------------
        qt_t = [pers.tile([P, L], BF16, tag=f"qt{p}", name=f"qt{p}") for p in range(2)]
        kt_t = [pers.tile([P, L], BF16, tag=f"kt{p}", name=f"kt{p}") for p in range(2)]
        vx_t = [pers.tile([P, NKB, 2, DH + 1], BF16, tag=f"vx{p}", name=f"vx{p}")
                for p in range(2)]
        osl_t = [pers.tile([P, NKB, P], BF16, tag=f"osl{p}", name=f"osl{p}") for p in range(2)]
        otT_t = [pers.tile([P, NKB, P], BF16, tag=f"otT{p}", name=f"otT{p}") for p in range(2)]

        for _it in range(iters):
            for pair in range(2):
                nc.gpsimd.memset(vx_t[pair][:], WS)

            # (w-term, x-term) for the compensated product
            TERMS = ((0, xh_sb), (1, xh_sb), (0, xl_sb))

            def qk_proj(pair, which, ci):
                w_sb, b_sb, dst = (
                    (wq_sb, bq_sb, qt_t[pair]) if which == 0
                    else (wk_sb, bk_sb, kt_t[pair]))
                sl = slice(ci * LQB, (ci + 1) * LQB)
                acc = ps.tile([P, LQB], F32, tag="acc", bufs=2)
                n = 0
                for wt, x_sb in TERMS:
                    for g2 in range(4):
                        nc.tensor.matmul(
                            acc[:],
                            w_sb[:, wt, g2, :, pair, :],
                            x_sb[:, 2 * g2:2 * g2 + 2, sl],
                            start=(n == 0), stop=(n == 11), perf_mode=DR)
                        n += 1
                nc.vector.tensor_scalar_add(dst[:, sl], acc[:],
                                            b_sb[:, pair:pair + 1])

            def v_proj(j):
                acc = ps.tile([P, 2 * P], F32, tag="acc", bufs=2)
                jsl = slice(j * P, (j + 1) * P)
                n = 0
                for wt, x_sb in TERMS:
                    for g2 in range(4):
                        nc.tensor.matmul(
                            acc[:],
                            x_sb[:, 2 * g2:2 * g2 + 2, jsl],
                            wv_sb[:, wt, g2, :, :],
                            start=(n == 0), stop=(n == 11), perf_mode=DR)
                        n += 1
                for pair in range(2):
                    for h in range(2):
                        c0 = pair * P + h * DH
                        nc.vector.tensor_add(
                            vx_t[pair][:, j, h, 0:DH],
                            acc[:, c0:c0 + DH], bv_bc[:, c0:c0 + DH])

            def s_group(pair, h, ci, jp):
                """S matmuls + exp (+ masks) for pair-tile jp; returns pt."""
                qt, kt = qt_t[pair], kt_t[pair]
                hp = h * DH
                s = ps.tile([P, 2, LQB], F32, tag="s", bufs=2)
                pt = work.tile([P, 2, LQB], BF16, tag="pt", bufs=8)
                poff = 0 if jp <= 2 * ci else 2 * P
                for jj in range(2):
                    j = 2 * jp + jj
                    off = poff if jp >= 2 * ci else 0
                    nc.tensor.matmul(
                        s[:, jj, off:LQB],
                        kt[hp:hp + DH, j * P:(j + 1) * P],
                        qt[hp:hp + DH, ci * LQB + off:(ci + 1) * LQB],
                        start=True, stop=True)
                if jp < 2 * ci:
                    nc.scalar.activation(pt[:], s[:], AF.Exp, scale=SC)
                else:
                    nc.scalar.activation(pt[:, :, poff:LQB],
                                         s[:, :, poff:LQB], AF.Exp, scale=SC)
                    for jj in range(2):
                        m = 2 * jp + jj - 4 * ci
                        if m >= 0:
                            nc.vector.tensor_mul(
                                pt[:, jj, poff:LQB], pt[:, jj, poff:LQB],
                                mask_sb[:, m, poff:LQB])
                return pt

            def pv_group(pair, h, ci, jp, pt, ot):
                # ot is one PSUM bank: hardware start zeroes the whole bank,
                # so the (head, chunk) group has exactly one start (first
                # matmul) and one stop (last matmul).
                vx = vx_t[pair]
                for jj in range(2):
                    j = 2 * jp + jj
                    m = j - 4 * ci
                    for sb in range(max(0, m), 4):
                        nc.tensor.matmul(
                            ot[:, sb, 0:DH + 1],
                            pt[:, jj, sb * P:(sb + 1) * P],
                            vx[:, j, h, :],
                            start=(j == 0 and sb == 0),
                            stop=(j == 4 * ci + 3 and sb == 3),
                            skip_group_check=True)

            def normalize(pair, h, ci, ot):
                hp = h * DH
                rec = work.tile([P, 4], F32, tag="rec", bufs=4)
                nc.vector.reciprocal(rec[:], ot[:, :, DH])
                for sb in range(4):
                    nc.vector.tensor_scalar_mul(
                        osl_t[pair][:, 4 * ci + sb, hp:hp + DH],
                        ot[:, sb, 0:DH], rec[:, sb:sb + 1])

            def outproj_half(lb, half, osb):
                acc = ps.tile([P, LQB], F32, tag="acc", bufs=2)
                for pair in range(2):
                    nc.tensor.matmul(
                        acc[:],
                        otT_t[pair][:, lb, :],
                        wo_sb[:, pair, half * LQB:(half + 1) * LQB],
                        start=(pair == 0), stop=(pair == 1))
                nc.vector.tensor_copy(osb[:, half * LQB:(half + 1) * LQB],
                                      acc[:])

            def outproj_units(ci):
                units = []
                for lb in range(4 * ci, 4 * ci + 4):
                    osb = work.tile([P, D], BF16, tag="osb", bufs=4,
                                    name=f"osb{lb}")
                    for half in range(2):
                        def unit(l=lb, o=osb, hf=half):
                            outproj_half(l, hf, o)
                            nc.sync.dma_start(
                                out=out[l * P:(l + 1) * P,
                                        hf * LQB:(hf + 1) * LQB],
                                in_=o[:, hf * LQB:(hf + 1) * LQB])
                        units.append(unit)
                return units

            # prologue: chunk 0 projections for pair 0 only; pair 1 comes
            # through the filler queue during pair-0 attention. Q and K are
            # staged hi-terms-first so K's hi matmuls overlap the xl DMA.
            pro_accs = []
            for which in range(2):
                w_sb = (wq_sb, wk_sb)[which]
                acc = ps.tile([P, LQB], F32, tag="acc", bufs=2,
                              name=f"proacc{which}")
                for g2 in range(4):
                    nc.tensor.matmul(
                        acc[:], w_sb[:, 0, g2, :, 0, :],
                        xh_sb[:, 2 * g2:2 * g2 + 2, 0:LQB],
                        start=(g2 == 0), stop=False, perf_mode=DR)
                pro_accs.append(acc)
            for which in range(2):
                w_sb, b_sb, dst = ((wq_sb, bq_sb, qt_t[0]),
                                   (wk_sb, bk_sb, kt_t[0]))[which]
                acc = pro_accs[which]
                n = 0
                for wt, x_sb in ((1, xh_sb), (0, xl_sb)):
                    for g2 in range(4):
                        nc.tensor.matmul(
                            acc[:], w_sb[:, wt, g2, :, 0, :],
                            x_sb[:, 2 * g2:2 * g2 + 2, 0:LQB],
                            start=False, stop=(n == 7), perf_mode=DR)
                        n += 1
                nc.vector.tensor_scalar_add(dst[:, 0:LQB], acc[:],
                                            b_sb[:, 0:1])
            for j in range(4):
                v_proj(j)

            # steady state: per chunk, the two head-streams of each pair are
            # interleaved at pair-tile granularity with PV pipelined one
            # round behind S, and a filler queue (next-chunk projections,
            # prev-chunk output projection) feeds the PE stream's exp-wait
            # windows.
            fillers = [lambda: qk_proj(1, 0, 0), lambda: qk_proj(1, 1, 0)]
            deferred = []
            quota = [0.0]

            def drain(slots_left, rate=1.0):
                # spread remaining fillers over remaining drain slots; rate>1
                # front-loads (for units with a chunk-boundary deadline)
                quota[0] += rate * len(fillers) / max(1.0, slots_left)
                while quota[0] >= 1.0 and fillers:
                    quota[0] -= 1.0
                    fillers.pop(0)()

            for ci in range(NLQ):
                nxt = ci + 1
                if nxt < NLQ:
                    for pair in range(2):
                        fillers.append(lambda p=pair: qk_proj(p, 0, nxt))
                        fillers.append(lambda p=pair: qk_proj(p, 1, nxt))
                    for j in range(4 * nxt, 4 * nxt + 4):
                        fillers.append(lambda jj=j: v_proj(jj))
                if ci == NLQ - 1:
                    # late chunks are exp-bound and filler-starved: feed them
                    # the deferred output-projection units
                    fillers.extend(deferred)
                    deferred = []
                nrounds = 2 * ci + 2
                slots = 4 * nrounds
                for pair in range(2):
                    ot_h = [ps.tile([P, 4, P], F32, tag="ot", name=f"ot{h}", bufs=2)
                            for h in range(2)]
                    pt_prev = [None, None]
                    for jp in range(nrounds):
                        for h in range(2):
                            pt = s_group(pair, h, ci, jp)
                            if pt_prev[h] is not None:
                                pv_group(pair, h, ci, jp - 1, pt_prev[h],
                                         ot_h[h])
                            pt_prev[h] = pt
                            drain(slots, 1.0)
                            slots -= 1
                    for h in range(2):
                        pv_group(pair, h, ci, nrounds - 1, pt_prev[h], ot_h[h])
                        normalize(pair, h, ci, ot_h[h])
                    for qb in range(4 * ci, 4 * ci + 2):
                        nc.sync.dma_start(out=otT_t[pair][:, qb, :],
                                          in_=osl_t[pair][:, qb, :],
                                          transpose=True)
                    nc.sync.dma_start(
                        out=otT_t[pair][:, 4 * ci + 2:4 * ci + 4, :],
                        in_=osl_t[pair][:, 4 * ci + 2:4 * ci + 4, :],
                        transpose=True)
                while fillers:
                    fillers.pop(0)()
                if ci >= 2:
                    fillers.extend(outproj_units(ci))
                else:
                    deferred.extend(outproj_units(ci))
            while fillers:
                fillers.pop(0)()

        if dbg:
            for name_, src, dst in (("qt", qt_t[0], dbg_qt),
                                    ("kt", kt_t[0], dbg_kt),
                                    ("vx", vx_t[0], dbg_vx),
                                    ("osl", osl_t[0], dbg_osl),
                                    ("otT", otT_t[0], dbg_otT)):
                tmp = work.tile(list(src.shape), F32, tag=f"dbg{name_}",
                                name=f"dbg{name_}")
                nc.vector.tensor_copy(tmp[:], src[:])
                nc.sync.dma_start(out=dst, in_=tmp[:])

    nc.compile()
    return nc


_CACHE = {}


def _get_nc(mm_dt=None, iters=1):
    key = iters
    if key not in _CACHE:
        _CACHE[key] = build_module(iters)
    return _CACHE[key]


def _split_fp8(a):
    hi = a.astype(E4M3)
    lo = (a - hi.astype(np.float32)).astype(E4M3)
    return hi, lo


def _make_in_maps(x, causal_mask, wq, bq, wk, bk, wv, bv, wo):
    x = np.asarray(x, np.float32)
    cm = np.asarray(causal_mask)
    # mask tile m (for k-block j = 4i+m): keep[p, c] = (c >= 128m + p)
    mt = np.empty((P, 4, LQB), np.float32)
    for m in range(4):
        mt[:, m, :] = (~cm[0, 0, 0:LQB, m * P:(m + 1) * P]).T
    mt = mt.astype(BF16NP)

    wq = np.asarray(wq, np.float32)
    wk = np.asarray(wk, np.float32)
    wv = np.asarray(wv, np.float32)
    wo = np.asarray(wo, np.float32)
    bq = np.asarray(bq, np.float32)
    bk = np.asarray(bk, np.float32)
    bv = np.asarray(bv, np.float32)

    in_maps = []
    for c in range(N_CORES):
        b = c // 4
        g = c % 4
        cols = slice(256 * g, 256 * (g + 1))

        xt = np.ascontiguousarray(
            x[b].T.reshape(KD, P, L).transpose(1, 0, 2))
        xhi, xlo = _split_fp8(xt)

        def pack_qk(w):
            # [D, 256] -> [p, g2, slot, pair, m], scaled
            a = (w[:, cols] * WS).reshape(4, 2, P, 2, P).transpose(2, 0, 1, 3, 4)
            hi, lo = _split_fp8(np.ascontiguousarray(a))
            return np.stack([hi, lo])

        def pack_v(w):
            a = (w[:, cols] * WS).reshape(4, 2, P, 2 * P).transpose(2, 0, 1, 3)
            hi, lo = _split_fp8(np.ascontiguousarray(a))
            return np.stack([hi, lo])

        in_maps.append({
            "xh": xhi,
            "xl": xlo,
            "wq8": pack_qk(wq),
            "wk8": pack_qk(wk),
            "wv8": pack_v(wv),
            "wo": np.ascontiguousarray(
                wo[cols, :].reshape(2, P, D)).astype(BF16NP),
            "bq": np.ascontiguousarray((bq[cols] * WS).reshape(2, P).T),
            "bk": np.ascontiguousarray((bk[cols] * WS).reshape(2, P).T),
            "bv": np.ascontiguousarray(bv[cols] * WS),
            "mask": mt,
        })
    return in_maps


def run(inputs, trace=False, mm_dt=None, iters=1, **kw):
    nc = _get_nc(mm_dt, iters)
    in_maps = _make_in_maps(
        inputs["x"], inputs["causal_mask"], inputs["wq"], inputs["bq"],
        inputs["wk"], inputs["bk"], inputs["wv"], inputs["bv"], inputs["wo"])
    res = run_bass_kernel_spmd(nc, in_maps, list(range(N_CORES)),
                               trace=trace, **kw)
    bo = np.asarray(inputs["bo"], np.float32)
    out = np.zeros((B, L, D), np.float32)
    for c in range(N_CORES):
        out[c // 4] += res.results[c]["out"].astype(np.float32)
    out += bo[None, None, :]
    return out, res


def kernel(**inputs):
    out, _ = run(inputs)
    return out


# revision 35
# speedup vs baseline: 1.0051x; 1.0051x over previous
"""Multi-head attention (B=2, L=2048, D=1024, H=16) on 8 TRN2 NeuronCores.

Sharding: core c handles batch b=c//4 and heads 4*(c%4) .. 4*(c%4)+3
(tensor-parallel over heads x data-parallel over batch). Each core computes a
partial [L, D] output (its heads' contribution through wo); the host sums the
4 partials per batch and adds bo.

Device-side design:
  - QKV projections run in fp8-e4m3 DoubleRow matmuls (0.5 cyc/row, two
    128-row contractions per instruction). Accuracy is preserved with a
    compensated split prepared on the host: x = xh + xl, w = wh + wl (wl/xl
    are the fp8 quantization residuals), and x@w is computed as
    xh@wh + xl@wh + xh@wl (the dropped xl@wl term is ~1e-3 relative).
    Weights are pre-scaled by 32 so their uniform(-1/32,1/32) range stays in
    fp8 normal range; biases are pre-scaled to match, and the V "ones"
    column carries the same scale so softmax normalization cancels it.
  - Q^T/K^T [2*Dh, L]; S^T [k, q] blocks of [128, 512] with exp on paired
    2-bank PSUM tiles; causal handled by skipping k-blocks above the
    diagonal, shrinking diagonal tiles to their unmasked column range, and
    bf16 multiplicative mask tiles for the intra-block triangles.
  - PV runs transposed-back: O [q, dh] via lhsT = P^T slice (full 128-wide
    stationary, N=65 moving V+ones) which halves PV row count vs the
    [dh, q] orientation and makes the softmax denominator per-partition
    (one reciprocal + tensor_scalar per q-subblock, no PE broadcast).
  - O [q, dh] -> O^T via XBAR DMA transposes (SBUF->SBUF, 14ns/tile).
  - Projections/attention/output are software-pipelined per 512-row q-chunk:
    within a chunk the two head-streams of a pair interleave at pair-tile
    granularity with PV one round behind S (hiding exp latency), and a
    filler queue spreads next-chunk projections and deferred output
    projections into the PE stream's exp-wait windows. Output-projection
    units of early chunks are deferred to the exp-bound late chunks, which
    would otherwise starve the PE. The partial output is written bf16 (the
    host accumulates cores in f32).
"""

from contextlib import ExitStack

import numpy as np
import ml_dtypes

import concourse.bass as bass
import concourse.mybir as mybir
import concourse.tile as tile
from concourse import bacc
from concourse.bass_utils import run_bass_kernel_spmd

B, L, D, H = 2, 2048, 1024, 16
DH = D // H          # 64
P = 128              # partitions
NPAIR = 2            # head pairs per core (4 heads)
LQB = 512            # q chunk
NLQ = L // LQB       # 4
NKB = L // P         # 16 k blocks
KD = D // P          # 8 contraction blocks over D
N_CORES = 8
WS = 32.0            # host-side weight scale (fp8 range)
SC = (1.0 / np.sqrt(DH)) / (WS * WS)   # exp scale: undo WS^2 in scores

F32 = mybir.dt.float32
BF16 = mybir.dt.bfloat16
FP8 = mybir.dt.float8e4
AF = mybir.ActivationFunctionType
DR = mybir.MatmulPerfMode.DoubleRow
E4M3 = ml_dtypes.float8_e4m3
BF16NP = ml_dtypes.bfloat16


def build_module(iters=1, dbg=False):
    nc = bacc.Bacc("TRN2", target_bir_lowering=False, debug=False,
                   num_devices=N_CORES)
    if dbg:
        dbg_qt = nc.dram_tensor("dbg_qt", [P, L], F32, kind="ExternalOutput").ap()
        dbg_kt = nc.dram_tensor("dbg_kt", [P, L], F32, kind="ExternalOutput").ap()
        dbg_vx = nc.dram_tensor("dbg_vx", [P, NKB, 2, DH + 1], F32,
                                kind="ExternalOutput").ap()
        dbg_osl = nc.dram_tensor("dbg_osl", [P, NKB, P], F32,
                                 kind="ExternalOutput").ap()
        dbg_otT = nc.dram_tensor("dbg_otT", [P, NKB, P], F32,
                                 kind="ExternalOutput").ap()

    xh_d = nc.dram_tensor("xh", [P, KD, L], FP8, kind="ExternalInput").ap()
    xl_d = nc.dram_tensor("xl", [P, KD, L], FP8, kind="ExternalInput").ap()
    # [hl, p, g2, slot, pair, m]
    wq_d = nc.dram_tensor("wq8", [2, P, 4, 2, 2, P], FP8, kind="ExternalInput").ap()
    wk_d = nc.dram_tensor("wk8", [2, P, 4, 2, 2, P], FP8, kind="ExternalInput").ap()
    # [hl, p, g2, slot, ch]
    wv_d = nc.dram_tensor("wv8", [2, P, 4, 2, 2 * P], FP8, kind="ExternalInput").ap()
    wo_d = nc.dram_tensor("wo", [2, P, D], BF16, kind="ExternalInput").ap()
    bq_d = nc.dram_tensor("bq", [P, 2], F32, kind="ExternalInput").ap()
    bk_d = nc.dram_tensor("bk", [P, 2], F32, kind="ExternalInput").ap()
    bv_d = nc.dram_tensor("bv", [2 * P], F32, kind="ExternalInput").ap()
    mask_d = nc.dram_tensor("mask", [P, 4, LQB], BF16, kind="ExternalInput").ap()
    out = nc.dram_tensor("out", [L, D], BF16, kind="ExternalOutput").ap()

    with tile.TileContext(nc) as tc, ExitStack() as ctx:
        ctx.enter_context(
            nc.allow_low_precision(reason="fp8/bf16 matmul data path"))
        consts = ctx.enter_context(tc.tile_pool(name="consts", bufs=1))
        pers = ctx.enter_context(tc.tile_pool(name="pers", bufs=1))
        work = ctx.enter_context(tc.tile_pool(name="work", bufs=1))
        ps = ctx.enter_context(tc.tile_pool(name="ps", bufs=1, space="PSUM"))

        # ---- const tiles ---------------------------------------------------
        xh_sb = consts.tile([P, KD, L], FP8, tag="xh")
        xl_sb = consts.tile([P, KD, L], FP8, tag="xl")
        wq_sb = consts.tile([P, 2, 4, 2, 2, P], FP8, tag="wq")
        wk_sb = consts.tile([P, 2, 4, 2, 2, P], FP8, tag="wk")
        wv_sb = consts.tile([P, 2, 4, 2, 2 * P], FP8, tag="wv")
        wo_sb = consts.tile([P, 2, D], BF16, tag="wo")
        bq_sb = consts.tile([P, 2], F32, tag="bq")
        bk_sb = consts.tile([P, 2], F32, tag="bk")
        bv_bc = consts.tile([P, 2 * P], F32, tag="bv")
        mask_sb = consts.tile([P, 4, LQB], BF16, tag="mask")

        # DMA order matters: weights for chunk-0 QK first, then x chunk
        # slabs interleaved with the tensors each chunk unlocks.
        def load_x_chunk(ci, hl):
            sl = slice(ci * LQB, (ci + 1) * LQB)
            src = (xh_d, xl_d)[hl]
            dst = (xh_sb, xl_sb)[hl]
            nc.sync.dma_start(out=dst[:, :, sl], in_=src[:, :, sl])

        nc.sync.dma_start(out=wq_sb[:, 0], in_=wq_d[0])
        load_x_chunk(0, 0)
        nc.sync.dma_start(out=wk_sb[:, 0], in_=wk_d[0])
        nc.sync.dma_start(out=bq_sb[:], in_=bq_d)
        nc.sync.dma_start(out=bk_sb[:], in_=bk_d)
        load_x_chunk(0, 1)
        nc.sync.dma_start(out=wq_sb[:, 1], in_=wq_d[1])
        nc.sync.dma_start(out=wk_sb[:, 1], in_=wk_d[1])
        nc.sync.dma_start(out=wv_sb[:, 0], in_=wv_d[0])
        nc.sync.dma_start(out=wv_sb[:, 1], in_=wv_d[1])
        bv_b = bass.AP(tensor=bv_d.tensor, offset=bv_d.offset,
                       ap=[[0, P]] + list(bv_d.ap))
        nc.gpsimd.dma_start(out=bv_bc[:], in_=bv_b)
        nc.sync.dma_start(out=mask_sb[:], in_=mask_d)
        load_x_chunk(1, 0)
        load_x_chunk(1, 1)
        for pair in range(2):
            nc.sync.dma_start(out=wo_sb[:, pair], in_=wo_d[pair])
        for ci in range(2, NLQ):
            load_x_chunk(ci, 0)
            load_x_chunk(ci, 1)

        # ---- persistent work tiles ----------------------------------------
        qt_t = [pers.tile([P, L], BF16, tag=f"qt{p}", name=f"qt{p}") for p in range(2)]
        kt_t = [pers.tile([P, L], BF16, tag=f"kt{p}", name=f"kt{p}") for p in range(2)]
        vx_t = [pers.tile([P, NKB, 2, DH + 1], BF16, tag=f"vx{p}", name=f"vx{p}")
                for p in range(2)]
        osl_t = [pers.tile([P, NKB, P], BF16, tag=f"osl{p}", name=f"osl{p}") for p in range(2)]
        otT_t = [pers.tile([P, NKB, P], BF16, tag=f"otT{p}", name=f"otT{p}") for p in range(2)]

        for _it in range(iters):
            for pair in range(2):
                nc.gpsimd.memset(vx_t[pair][:], WS)

            # (w-term, x-term) for the compensated product
            TERMS = ((0, xh_sb), (1, xh_sb), (0, xl_sb))

            def qk_proj(pair, which, ci):
                w_sb, b_sb, dst = (
                    (wq_sb, bq_sb, qt_t[pair]) if which == 0
                    else (wk_sb, bk_sb, kt_t[pair]))
                sl = slice(ci * LQB, (ci + 1) * LQB)
                acc = ps.tile([P, LQB], F32, tag="acc", bufs=2)
                n = 0
                for wt, x_sb in TERMS:
                    for g2 in range(4):
                        nc.tensor.matmul(
                            acc[:],
                            w_sb[:, wt, g2, :, pair, :],
                            x_sb[:, 2 * g2:2 * g2 + 2, sl],
                            start=(n == 0), stop=(n == 11), perf_mode=DR)
                        n += 1
                nc.vector.tensor_scalar_add(dst[:, sl], acc[:],
                                            b_sb[:, pair:pair + 1])

            def v_proj(j):
                acc = ps.tile([P, 2 * P], F32, tag="acc", bufs=2)
                jsl = slice(j * P, (j + 1) * P)
                n = 0
                for wt, x_sb in TERMS:
                    for g2 in range(4):
                        nc.tensor.matmul(
                            acc[:],
                            x_sb[:, 2 * g2:2 * g2 + 2, jsl],
                            wv_sb[:, wt, g2, :, :],
                            start=(n == 0), stop=(n == 11), perf_mode=DR)
                        n += 1
                for pair in range(2):
                    for h in range(2):
                        c0 = pair * P + h * DH
                        nc.vector.tensor_add(
                            vx_t[pair][:, j, h, 0:DH],
                            acc[:, c0:c0 + DH], bv_bc[:, c0:c0 + DH])

            def s_group(pair, h, ci, jp):
                """S matmuls + exp (+ masks) for pair-tile jp; returns pt."""
                qt, kt = qt_t[pair], kt_t[pair]
                hp = h * DH
                s = ps.tile([P, 2, LQB], F32, tag="s", bufs=2)
                pt = work.tile([P, 2, LQB], BF16, tag="pt", bufs=8)
                poff = 0 if jp <= 2 * ci else 2 * P
                for jj in range(2):
                    j = 2 * jp + jj
                    off = poff if jp >= 2 * ci else 0
                    nc.tensor.matmul(
                        s[:, jj, off:LQB],
                        kt[hp:hp + DH, j * P:(j + 1) * P],
                        qt[hp:hp + DH, ci * LQB + off:(ci + 1) * LQB],
                        start=True, stop=True)
                if jp < 2 * ci:
                    nc.scalar.activation(pt[:], s[:], AF.Exp, scale=SC)
                else:
                    nc.scalar.activation(pt[:, :, poff:LQB],
                                         s[:, :, poff:LQB], AF.Exp, scale=SC)
                    for jj in range(2):
                        m = 2 * jp + jj - 4 * ci
                        if m >= 0:
                            nc.vector.tensor_mul(
                                pt[:, jj, poff:LQB], pt[:, jj, poff:LQB],
                                mask_sb[:, m, poff:LQB])
                return pt

            def pv_group(pair, h, ci, jp, pt, ot):
                # ot is one PSUM bank: hardware start zeroes the whole bank,
                # so the (head, chunk) group has exactly one start (first
                # matmul) and one stop (last matmul).
                vx = vx_t[pair]
                for jj in range(2):
                    j = 2 * jp + jj
                    m = j - 4 * ci
                    for sb in range(max(0, m), 4):
                        nc.tensor.matmul(
                            ot[:, sb, 0:DH + 1],
                            pt[:, jj, sb * P:(sb + 1) * P],
                            vx[:, j, h, :],
                            start=(j == 0 and sb == 0),
                            stop=(j == 4 * ci + 3 and sb == 3),
                            skip_group_check=True)

            def normalize(pair, h, ci, ot):
                hp = h * DH
                rec = work.tile([P, 4], F32, tag="rec", bufs=4)
                nc.vector.reciprocal(rec[:], ot[:, :, DH])
                for sb in range(4):
                    nc.vector.tensor_scalar_mul(
                        osl_t[pair][:, 4 * ci + sb, hp:hp + DH],
                        ot[:, sb, 0:DH], rec[:, sb:sb + 1])

            def outproj_half(lb, half, osb):
                acc = ps.tile([P, LQB], F32, tag="acc", bufs=2)
                for pair in range(2):
                    nc.tensor.matmul(
                        acc[:],
                        otT_t[pair][:, lb, :],
                        wo_sb[:, pair, half * LQB:(half + 1) * LQB],
                        start=(pair == 0), stop=(pair == 1))
                nc.vector.tensor_copy(osb[:, half * LQB:(half + 1) * LQB],
                                      acc[:])

            def outproj_units(ci):
                units = []
                for lb in range(4 * ci, 4 * ci + 4):
                    osb = work.tile([P, D], BF16, tag="osb", bufs=4,
                                    name=f"osb{lb}")
                    for half in range(2):
                        def unit(l=lb, o=osb, hf=half):
                            outproj_half(l, hf, o)
                            nc.sync.dma_start(
                                out=out[l * P:(l + 1) * P,
                                        hf * LQB:(hf + 1) * LQB],
                                in_=o[:, hf * LQB:(hf + 1) * LQB])
                        units.append(unit)
                return units

            # prologue: chunk 0 projections for pair 0 only; pair 1 comes
            # through the filler queue during pair-0 attention. Q and K are
            # staged hi-terms-first so K's hi matmuls overlap the xl DMA.
            pro_accs = []
            for which in range(2):
                w_sb = (wq_sb, wk_sb)[which]
                acc = ps.tile([P, LQB], F32, tag="acc", bufs=2,
                              name=f"proacc{which}")
                for g2 in range(4):
                    nc.tensor.matmul(
                        acc[:], w_sb[:, 0, g2, :, 0, :],
                        xh_sb[:, 2 * g2:2 * g2 + 2, 0:LQB],
                        start=(g2 == 0), stop=False, perf_mode=DR)
                pro_accs.append(acc)
            for which in range(2):
                w_sb, b_sb, dst = ((wq_sb, bq_sb, qt_t[0]),
                                   (wk_sb, bk_sb, kt_t[0]))[which]
                acc = pro_accs[which]
                n = 0
                for wt, x_sb in ((1, xh_sb), (0, xl_sb)):
                    for g2 in range(4):
                        nc.tensor.matmul(
                            acc[:], w_sb[:, wt, g2, :, 0, :],
                            x_sb[:, 2 * g2:2 * g2 + 2, 0:LQB],
                            start=False, stop=(n == 7), perf_mode=DR)
                        n += 1
                nc.vector.tensor_scalar_add(dst[:, 0:LQB], acc[:],
                                            b_sb[:, 0:1])
            for j in range(4):
                v_proj(j)

            # steady state: per chunk, the two head-streams of each pair are
            # interleaved at pair-tile granularity with PV pipelined one
            # round behind S, and a filler queue (next-chunk projections,
            # prev-chunk output projection) feeds the PE stream's exp-wait
            # windows.
            fillers = [lambda: qk_proj(1, 0, 0), lambda: qk_proj(1, 1, 0)]
            deferred = []
            quota = [0.0]

            def drain(slots_left, rate=1.0):
                # spread remaining fillers over remaining drain slots; rate>1
                # front-loads (for units with a chunk-boundary deadline)
                quota[0] += rate * len(fillers) / max(1.0, slots_left)
                while quota[0] >= 1.0 and fillers:
                    quota[0] -= 1.0
                    fillers.pop(0)()

            for ci in range(NLQ):
                nxt = ci + 1
                if nxt < NLQ:
                    for pair in range(2):
                        fillers.append(lambda p=pair: qk_proj(p, 0, nxt))
                        fillers.append(lambda p=pair: qk_proj(p, 1, nxt))
                    for j in range(4 * nxt, 4 * nxt + 4):
                        fillers.append(lambda jj=j: v_proj(jj))
                if ci == NLQ - 1:
                    # late chunks are exp-bound and filler-starved: feed them
                    # the deferred output-projection units
                    fillers.extend(deferred)
                    deferred = []
                nrounds = 2 * ci + 2
                slots = 4 * nrounds
                for pair in range(2):
                    ot_h = [ps.tile([P, 4, P], F32, tag="ot", name=f"ot{h}", bufs=2)
                            for h in range(2)]
                    pt_prev = [None, None]
                    for jp in range(nrounds):
                        for h in ((0, 1) if jp % 2 == 0 else (1, 0)):
                            pt = s_group(pair, h, ci, jp)
                            if pt_prev[h] is not None:
                                pv_group(pair, h, ci, jp - 1, pt_prev[h],
                                         ot_h[h])
                            pt_prev[h] = pt
                            drain(slots, 1.0)
                            slots -= 1
                    for h in range(2):
                        pv_group(pair, h, ci, nrounds - 1, pt_prev[h], ot_h[h])
                        normalize(pair, h, ci, ot_h[h])
                    for qb in range(4 * ci, 4 * ci + 2):
                        nc.sync.dma_start(out=otT_t[pair][:, qb, :],
                                          in_=osl_t[pair][:, qb, :],
                                          transpose=True)
                    nc.sync.dma_start(
                        out=otT_t[pair][:, 4 * ci + 2:4 * ci + 4, :],
                        in_=osl_t[pair][:, 4 * ci + 2:4 * ci + 4, :],
                        transpose=True)
                while fillers:
                    fillers.pop(0)()
                if ci >= 2:
                    fillers.extend(outproj_units(ci))
                else:
                    deferred.extend(outproj_units(ci))
            while fillers:
                fillers.pop(0)()

        if dbg:
            for name_, src, dst in (("qt", qt_t[0], dbg_qt),
                                    ("kt", kt_t[0], dbg_kt),
                                    ("vx", vx_t[0], dbg_vx),
                                    ("osl", osl_t[0], dbg_osl),
                                    ("otT", otT_t[0], dbg_otT)):
                tmp = work.tile(list(src.shape), F32, tag=f"dbg{name_}",
                                name=f"dbg{name_}")
                nc.vector.tensor_copy(tmp[:], src[:])
                nc.sync.dma_start(out=dst, in_=tmp[:])

    nc.compile()
    return nc


_CACHE = {}


def _get_nc(mm_dt=None, iters=1):
    key = iters
    if key not in _CACHE:
        _CACHE[key] = build_module(iters)
    return _CACHE[key]


def _split_fp8(a):
    hi = a.astype(E4M3)
    lo = (a - hi.astype(np.float32)).astype(E4M3)
    return hi, lo


def _make_in_maps(x, causal_mask, wq, bq, wk, bk, wv, bv, wo):
    x = np.asarray(x, np.float32)
    cm = np.asarray(causal_mask)
    # mask tile m (for k-block j = 4i+m): keep[p, c] = (c >= 128m + p)
    mt = np.empty((P, 4, LQB), np.float32)
    for m in range(4):
        mt[:, m, :] = (~cm[0, 0, 0:LQB, m * P:(m + 1) * P]).T
    mt = mt.astype(BF16NP)

    wq = np.asarray(wq, np.float32)
    wk = np.asarray(wk, np.float32)
    wv = np.asarray(wv, np.float32)
    wo = np.asarray(wo, np.float32)
    bq = np.asarray(bq, np.float32)
    bk = np.asarray(bk, np.float32)
    bv = np.asarray(bv, np.float32)

    in_maps = []
    for c in range(N_CORES):
        b = c // 4
        g = c % 4
        cols = slice(256 * g, 256 * (g + 1))

        xt = np.ascontiguousarray(
            x[b].T.reshape(KD, P, L).transpose(1, 0, 2))
        xhi, xlo = _split_fp8(xt)

        def pack_qk(w):
            # [D, 256] -> [p, g2, slot, pair, m], scaled
            a = (w[:, cols] * WS).reshape(4, 2, P, 2, P).transpose(2, 0, 1, 3, 4)
            hi, lo = _split_fp8(np.ascontiguousarray(a))
            return np.stack([hi, lo])

        def pack_v(w):
            a = (w[:, cols] * WS).reshape(4, 2, P, 2 * P).transpose(2, 0, 1, 3)
            hi, lo = _split_fp8(np.ascontiguousarray(a))
            return np.stack([hi, lo])

        in_maps.append({
            "xh": xhi,
            "xl": xlo,
            "wq8": pack_qk(wq),
            "wk8": pack_qk(wk),
            "wv8": pack_v(wv),
            "wo": np.ascontiguousarray(
                wo[cols, :].reshape(2, P, D)).astype(BF16NP),
            "bq": np.ascontiguousarray((bq[cols] * WS).reshape(2, P).T),
            "bk": np.ascontiguousarray((bk[cols] * WS).reshape(2, P).T),
            "bv": np.ascontiguousarray(bv[cols] * WS),
            "mask": mt,
        })
    return in_maps


def run(inputs, trace=False, mm_dt=None, iters=1, **kw):
    nc = _get_nc(mm_dt, iters)
    in_maps = _make_in_maps(
        inputs["x"], inputs["causal_mask"], inputs["wq"], inputs["bq"],
        inputs["wk"], inputs["bk"], inputs["wv"], inputs["bv"], inputs["wo"])
    res = run_bass_kernel_spmd(nc, in_maps, list(range(N_CORES)),
                               trace=trace, **kw)
    bo = np.asarray(inputs["bo"], np.float32)
    out = np.zeros((B, L, D), np.float32)
    for c in range(N_CORES):
        out[c // 4] += res.results[c]["out"].astype(np.float32)
    out += bo[None, None, :]
    return out, res


def kernel(**inputs):
    out, _ = run(inputs)
    return out


# revision 36
# speedup vs baseline: 1.0084x; 1.0033x over previous
"""Multi-head attention (B=2, L=2048, D=1024, H=16) on 8 TRN2 NeuronCores.

Sharding: core c handles batch b=c//4 and heads 4*(c%4) .. 4*(c%4)+3
(tensor-parallel over heads x data-parallel over batch). Each core computes a
partial [L, D] output (its heads' contribution through wo); the host sums the
4 partials per batch and adds bo.

Device-side design:
  - QKV projections run in fp8-e4m3 DoubleRow matmuls (0.5 cyc/row, two
    128-row contractions per instruction). Accuracy is preserved with a
    compensated split prepared on the host: x = xh + xl, w = wh + wl (wl/xl
    are the fp8 quantization residuals), and x@w is computed as
    xh@wh + xl@wh + xh@wl (the dropped xl@wl term is ~1e-3 relative).
    Weights are pre-scaled by 32 so their uniform(-1/32,1/32) range stays in
    fp8 normal range; biases are pre-scaled to match, and the V "ones"
    column carries the same scale so softmax normalization cancels it.
  - Q^T/K^T [2*Dh, L]; S^T [k, q] blocks of [128, 512] with exp on paired
    2-bank PSUM tiles; causal handled by skipping k-blocks above the
    diagonal, shrinking diagonal tiles to their unmasked column range, and
    bf16 multiplicative mask tiles for the intra-block triangles.
  - PV runs transposed-back: O [q, dh] via lhsT = P^T slice (full 128-wide
    stationary, N=65 moving V+ones) which halves PV row count vs the
    [dh, q] orientation and makes the softmax denominator per-partition
    (one reciprocal + tensor_scalar per q-subblock, no PE broadcast).
  - O [q, dh] -> O^T via XBAR DMA transposes (SBUF->SBUF, 14ns/tile).
  - Projections/attention/output are software-pipelined per 512-row q-chunk:
    within a chunk the two head-streams of a pair interleave at pair-tile
    granularity with PV one round behind S (hiding exp latency), and a
    filler queue spreads next-chunk projections and deferred output
    projections into the PE stream's exp-wait windows. Output-projection
    units of early chunks are deferred to the exp-bound late chunks, which
    would otherwise starve the PE. The partial output is written bf16 (the
    host accumulates cores in f32).
"""

from contextlib import ExitStack

import numpy as np
import ml_dtypes

import concourse.bass as bass
import concourse.mybir as mybir
import concourse.tile as tile
from concourse import bacc
from concourse.bass_utils import run_bass_kernel_spmd

B, L, D, H = 2, 2048, 1024, 16
DH = D // H          # 64
P = 128              # partitions
NPAIR = 2            # head pairs per core (4 heads)
LQB = 512            # q chunk
NLQ = L // LQB       # 4
NKB = L // P         # 16 k blocks
KD = D // P          # 8 contraction blocks over D
N_CORES = 8
WS = 32.0            # host-side weight scale (fp8 range)
SC = (1.0 / np.sqrt(DH)) / (WS * WS)   # exp scale: undo WS^2 in scores

F32 = mybir.dt.float32
BF16 = mybir.dt.bfloat16
FP8 = mybir.dt.float8e4
AF = mybir.ActivationFunctionType
DR = mybir.MatmulPerfMode.DoubleRow
E4M3 = ml_dtypes.float8_e4m3
BF16NP = ml_dtypes.bfloat16


def build_module(iters=1, dbg=False):
    nc = bacc.Bacc("TRN2", target_bir_lowering=False, debug=False,
                   num_devices=N_CORES)
    if dbg:
        dbg_qt = nc.dram_tensor("dbg_qt", [P, L], F32, kind="ExternalOutput").ap()
        dbg_kt = nc.dram_tensor("dbg_kt", [P, L], F32, kind="ExternalOutput").ap()
        dbg_vx = nc.dram_tensor("dbg_vx", [P, NKB, 2, DH + 1], F32,
                                kind="ExternalOutput").ap()
        dbg_osl = nc.dram_tensor("dbg_osl", [P, NKB, P], F32,
                                 kind="ExternalOutput").ap()
        dbg_otT = nc.dram_tensor("dbg_otT", [P, NKB, P], F32,
                                 kind="ExternalOutput").ap()

    xh_d = nc.dram_tensor("xh", [P, KD, L], FP8, kind="ExternalInput").ap()
    xl_d = nc.dram_tensor("xl", [P, KD, L], FP8, kind="ExternalInput").ap()
    # [hl, p, g2, slot, pair, m]
    wq_d = nc.dram_tensor("wq8", [2, P, 4, 2, 2, P], FP8, kind="ExternalInput").ap()
    wk_d = nc.dram_tensor("wk8", [2, P, 4, 2, 2, P], FP8, kind="ExternalInput").ap()
    # [hl, p, g2, slot, ch]
    wv_d = nc.dram_tensor("wv8", [2, P, 4, 2, 2 * P], FP8, kind="ExternalInput").ap()
    wo_d = nc.dram_tensor("wo", [2, P, D], BF16, kind="ExternalInput").ap()
    bq_d = nc.dram_tensor("bq", [P, 2], F32, kind="ExternalInput").ap()
    bk_d = nc.dram_tensor("bk", [P, 2], F32, kind="ExternalInput").ap()
    bv_d = nc.dram_tensor("bv", [2 * P], F32, kind="ExternalInput").ap()
    mask_d = nc.dram_tensor("mask", [P, 4, LQB], BF16, kind="ExternalInput").ap()
    out = nc.dram_tensor("out", [L, D], BF16, kind="ExternalOutput").ap()

    with tile.TileContext(nc) as tc, ExitStack() as ctx:
        ctx.enter_context(
            nc.allow_low_precision(reason="fp8/bf16 matmul data path"))
        consts = ctx.enter_context(tc.tile_pool(name="consts", bufs=1))
        pers = ctx.enter_context(tc.tile_pool(name="pers", bufs=1))
        work = ctx.enter_context(tc.tile_pool(name="work", bufs=1))
        ps = ctx.enter_context(tc.tile_pool(name="ps", bufs=1, space="PSUM"))

        # ---- const tiles ---------------------------------------------------
        xh_sb = consts.tile([P, KD, L], FP8, tag="xh")
        xl_sb = consts.tile([P, KD, L], FP8, tag="xl")
        wq_sb = consts.tile([P, 2, 4, 2, 2, P], FP8, tag="wq")
        wk_sb = consts.tile([P, 2, 4, 2, 2, P], FP8, tag="wk")
        wv_sb = consts.tile([P, 2, 4, 2, 2 * P], FP8, tag="wv")
        wo_sb = consts.tile([P, 2, D], BF16, tag="wo")
        bq_sb = consts.tile([P, 2], F32, tag="bq")
        bk_sb = consts.tile([P, 2], F32, tag="bk")
        bv_bc = consts.tile([P, 2 * P], F32, tag="bv")
        mask_sb = consts.tile([P, 4, LQB], BF16, tag="mask")

        # DMA order matters: weights for chunk-0 QK first, then x chunk
        # slabs interleaved with the tensors each chunk unlocks.
        def load_x_chunk(ci, hl):
            sl = slice(ci * LQB, (ci + 1) * LQB)
            src = (xh_d, xl_d)[hl]
            dst = (xh_sb, xl_sb)[hl]
            nc.sync.dma_start(out=dst[:, :, sl], in_=src[:, :, sl])

        nc.sync.dma_start(out=wq_sb[:, 0], in_=wq_d[0])
        load_x_chunk(0, 0)
        nc.sync.dma_start(out=wk_sb[:, 0], in_=wk_d[0])
        nc.sync.dma_start(out=bq_sb[:], in_=bq_d)
        nc.sync.dma_start(out=bk_sb[:], in_=bk_d)
        load_x_chunk(0, 1)
        nc.sync.dma_start(out=wq_sb[:, 1], in_=wq_d[1])
        nc.sync.dma_start(out=wk_sb[:, 1], in_=wk_d[1])
        nc.sync.dma_start(out=wv_sb[:, 0], in_=wv_d[0])
        nc.sync.dma_start(out=wv_sb[:, 1], in_=wv_d[1])
        bv_b = bass.AP(tensor=bv_d.tensor, offset=bv_d.offset,
                       ap=[[0, P]] + list(bv_d.ap))
        nc.gpsimd.dma_start(out=bv_bc[:], in_=bv_b)
        nc.sync.dma_start(out=mask_sb[:], in_=mask_d)
        load_x_chunk(1, 0)
        load_x_chunk(1, 1)
        for pair in range(2):
            nc.sync.dma_start(out=wo_sb[:, pair], in_=wo_d[pair])
        for ci in range(2, NLQ):
            load_x_chunk(ci, 0)
            load_x_chunk(ci, 1)

        # ---- persistent work tiles ----------------------------------------
        qt_t = [pers.tile([P, L], BF16, tag=f"qt{p}", name=f"qt{p}") for p in range(2)]
        kt_t = [pers.tile([P, L], BF16, tag=f"kt{p}", name=f"kt{p}") for p in range(2)]
        vx_t = [pers.tile([P, NKB, 2, DH + 1], BF16, tag=f"vx{p}", name=f"vx{p}")
                for p in range(2)]
        osl_t = [pers.tile([P, NKB, P], BF16, tag=f"osl{p}", name=f"osl{p}") for p in range(2)]
        otT_t = [pers.tile([P, NKB, P], BF16, tag=f"otT{p}", name=f"otT{p}") for p in range(2)]

        for _it in range(iters):
            for pair in range(2):
                nc.gpsimd.memset(vx_t[pair][:], WS)

            # (w-term, x-term) for the compensated product
            TERMS = ((0, xh_sb), (1, xh_sb), (0, xl_sb))

            def qk_proj(pair, which, ci):
                w_sb, b_sb, dst = (
                    (wq_sb, bq_sb, qt_t[pair]) if which == 0
                    else (wk_sb, bk_sb, kt_t[pair]))
                sl = slice(ci * LQB, (ci + 1) * LQB)
                acc = ps.tile([P, LQB], F32, tag="acc", bufs=2)
                n = 0
                for wt, x_sb in TERMS:
                    for g2 in range(4):
                        nc.tensor.matmul(
                            acc[:],
                            w_sb[:, wt, g2, :, pair, :],
                            x_sb[:, 2 * g2:2 * g2 + 2, sl],
                            start=(n == 0), stop=(n == 11), perf_mode=DR)
                        n += 1
                nc.vector.tensor_scalar_add(dst[:, sl], acc[:],
                                            b_sb[:, pair:pair + 1])

            def v_proj(j):
                acc = ps.tile([P, 2 * P], F32, tag="acc", bufs=2)
                jsl = slice(j * P, (j + 1) * P)
                n = 0
                for wt, x_sb in TERMS:
                    for g2 in range(4):
                        nc.tensor.matmul(
                            acc[:],
                            x_sb[:, 2 * g2:2 * g2 + 2, jsl],
                            wv_sb[:, wt, g2, :, :],
                            start=(n == 0), stop=(n == 11), perf_mode=DR)
                        n += 1
                for pair in range(2):
                    for h in range(2):
                        c0 = pair * P + h * DH
                        nc.vector.tensor_add(
                            vx_t[pair][:, j, h, 0:DH],
                            acc[:, c0:c0 + DH], bv_bc[:, c0:c0 + DH])

            def s_group(pair, h, ci, jp):
                """S matmuls + exp (+ masks) for pair-tile jp; returns pt."""
                qt, kt = qt_t[pair], kt_t[pair]
                hp = h * DH
                s = ps.tile([P, 2, LQB], F32, tag="s", bufs=2)
                pt = work.tile([P, 2, LQB], BF16, tag="pt", bufs=8)
                poff = 0 if jp <= 2 * ci else 2 * P
                for jj in range(2):
                    j = 2 * jp + jj
                    off = poff if jp >= 2 * ci else 0
                    nc.tensor.matmul(
                        s[:, jj, off:LQB],
                        kt[hp:hp + DH, j * P:(j + 1) * P],
                        qt[hp:hp + DH, ci * LQB + off:(ci + 1) * LQB],
                        start=True, stop=True)
                if jp < 2 * ci:
                    nc.scalar.activation(pt[:], s[:], AF.Exp, scale=SC)
                else:
                    nc.scalar.activation(pt[:, :, poff:LQB],
                                         s[:, :, poff:LQB], AF.Exp, scale=SC)
                    for jj in range(2):
                        m = 2 * jp + jj - 4 * ci
                        if m >= 0:
                            nc.vector.tensor_mul(
                                pt[:, jj, poff:LQB], pt[:, jj, poff:LQB],
                                mask_sb[:, m, poff:LQB])
                return pt

            def pv_group(pair, h, ci, jp, pt, ot):
                # ot is one PSUM bank: hardware start zeroes the whole bank,
                # so the (head, chunk) group has exactly one start (first
                # matmul) and one stop (last matmul).
                vx = vx_t[pair]
                for jj in range(2):
                    j = 2 * jp + jj
                    m = j - 4 * ci
                    for sb in range(max(0, m), 4):
                        nc.tensor.matmul(
                            ot[:, sb, 0:DH + 1],
                            pt[:, jj, sb * P:(sb + 1) * P],
                            vx[:, j, h, :],
                            start=(j == 0 and sb == 0),
                            stop=(j == 4 * ci + 3 and sb == 3),
                            skip_group_check=True)

            def normalize(pair, h, ci, ot):
                hp = h * DH
                rec = work.tile([P, 4], F32, tag="rec", bufs=4)
                nc.vector.reciprocal(rec[:], ot[:, :, DH])
                for sb in range(4):
                    nc.vector.tensor_scalar_mul(
                        osl_t[pair][:, 4 * ci + sb, hp:hp + DH],
                        ot[:, sb, 0:DH], rec[:, sb:sb + 1])

            def outproj_half(lb, half, osb):
                acc = ps.tile([P, LQB], F32, tag="acc", bufs=2)
                for pair in range(2):
                    nc.tensor.matmul(
                        acc[:],
                        otT_t[pair][:, lb, :],
                        wo_sb[:, pair, half * LQB:(half + 1) * LQB],
                        start=(pair == 0), stop=(pair == 1))
                nc.vector.tensor_copy(osb[:, half * LQB:(half + 1) * LQB],
                                      acc[:])

            def outproj_units(ci):
                units = []
                for lb in range(4 * ci, 4 * ci + 4):
                    osb = work.tile([P, D], BF16, tag="osb", bufs=4,
                                    name=f"osb{lb}")
                    for half in range(2):
                        def unit(l=lb, o=osb, hf=half):
                            outproj_half(l, hf, o)
                            nc.sync.dma_start(
                                out=out[l * P:(l + 1) * P,
                                        hf * LQB:(hf + 1) * LQB],
                                in_=o[:, hf * LQB:(hf + 1) * LQB])
                        units.append(unit)
                return units

            # prologue: chunk 0 projections for pair 0 only; pair 1 comes
            # through the filler queue during pair-0 attention. Q and K are
            # staged hi-terms-first so K's hi matmuls overlap the xl DMA.
            pro_accs = []
            for which in range(2):
                w_sb = (wq_sb, wk_sb)[which]
                acc = ps.tile([P, LQB], F32, tag="acc", bufs=2,
                              name=f"proacc{which}")
                for g2 in range(4):
                    nc.tensor.matmul(
                        acc[:], w_sb[:, 0, g2, :, 0, :],
                        xh_sb[:, 2 * g2:2 * g2 + 2, 0:LQB],
                        start=(g2 == 0), stop=False, perf_mode=DR)
                pro_accs.append(acc)
            for which in range(2):
                w_sb, b_sb, dst = ((wq_sb, bq_sb, qt_t[0]),
                                   (wk_sb, bk_sb, kt_t[0]))[which]
                acc = pro_accs[which]
                n = 0
                for wt, x_sb in ((1, xh_sb), (0, xl_sb)):
                    for g2 in range(4):
                        nc.tensor.matmul(
                            acc[:], w_sb[:, wt, g2, :, 0, :],
                            x_sb[:, 2 * g2:2 * g2 + 2, 0:LQB],
                            start=False, stop=(n == 7), perf_mode=DR)
                        n += 1
                nc.vector.tensor_scalar_add(dst[:, 0:LQB], acc[:],
                                            b_sb[:, 0:1])
            for j in range(4):
                v_proj(j)

            # steady state: per chunk, the two head-streams of each pair are
            # interleaved at pair-tile granularity with PV pipelined one
            # round behind S, and a filler queue (next-chunk projections,
            # prev-chunk output projection) feeds the PE stream's exp-wait
            # windows.
            fillers = [lambda: qk_proj(1, 0, 0), lambda: qk_proj(1, 1, 0)]
            deferred = []
            quota = [0.0]

            def drain(slots_left, rate=1.0):
                # spread remaining fillers over remaining drain slots; rate>1
                # front-loads (for units with a chunk-boundary deadline)
                quota[0] += rate * len(fillers) / max(1.0, slots_left)
                while quota[0] >= 1.0 and fillers:
                    quota[0] -= 1.0
                    fillers.pop(0)()

            for ci in range(NLQ):
                nxt = ci + 1
                if nxt < NLQ:
                    for pair in range(2):
                        fillers.append(lambda p=pair: qk_proj(p, 0, nxt))
                        fillers.append(lambda p=pair: qk_proj(p, 1, nxt))
                    for j in range(4 * nxt, 4 * nxt + 4):
                        fillers.append(lambda jj=j: v_proj(jj))
                if ci == NLQ - 1:
                    # late chunks are exp-bound and filler-starved: feed them
                    # the deferred output-projection units
                    fillers.extend(deferred)
                    deferred = []
                nrounds = 2 * ci + 2
                slots = 4 * nrounds
                for pair in range(2):
                    ot_h = [ps.tile([P, 4, P], F32, tag="ot", name=f"ot{h}", bufs=2)
                            for h in range(2)]
                    pt_prev = [None, None]
                    for jp in range(nrounds):
                        for h in ((0, 1) if (jp + pair) % 2 == 0 else (1, 0)):
                            pt = s_group(pair, h, ci, jp)
                            if pt_prev[h] is not None:
                                pv_group(pair, h, ci, jp - 1, pt_prev[h],
                                         ot_h[h])
                            pt_prev[h] = pt
                            drain(slots, 1.0)
                            slots -= 1
                    for h in range(2):
                        pv_group(pair, h, ci, nrounds - 1, pt_prev[h], ot_h[h])
                        normalize(pair, h, ci, ot_h[h])
                    for qb in range(4 * ci, 4 * ci + 2):
                        nc.sync.dma_start(out=otT_t[pair][:, qb, :],
                                          in_=osl_t[pair][:, qb, :],
                                          transpose=True)
                    nc.sync.dma_start(
                        out=otT_t[pair][:, 4 * ci + 2:4 * ci + 4, :],
                        in_=osl_t[pair][:, 4 * ci + 2:4 * ci + 4, :],
                        transpose=True)
                while fillers:
                    fillers.pop(0)()
                if ci >= 2:
                    fillers.extend(outproj_units(ci))
                else:
                    deferred.extend(outproj_units(ci))
            while fillers:
                fillers.pop(0)()

        if dbg:
            for name_, src, dst in (("qt", qt_t[0], dbg_qt),
                                    ("kt", kt_t[0], dbg_kt),
                                    ("vx", vx_t[0], dbg_vx),
                                    ("osl", osl_t[0], dbg_osl),
                                    ("otT", otT_t[0], dbg_otT)):
                tmp = work.tile(list(src.shape), F32, tag=f"dbg{name_}",
                                name=f"dbg{name_}")
                nc.vector.tensor_copy(tmp[:], src[:])
                nc.sync.dma_start(out=dst, in_=tmp[:])

    nc.compile()
    return nc


_CACHE = {}


def _get_nc(mm_dt=None, iters=1):
    key = iters
    if key not in _CACHE:
        _CACHE[key] = build_module(iters)
    return _CACHE[key]


def _split_fp8(a):
    hi = a.astype(E4M3)
    lo = (a - hi.astype(np.float32)).astype(E4M3)
    return hi, lo


def _make_in_maps(x, causal_mask, wq, bq, wk, bk, wv, bv, wo):
    x = np.asarray(x, np.float32)
    cm = np.asarray(causal_mask)
    # mask tile m (for k-block j = 4i+m): keep[p, c] = (c >= 128m + p)
    mt = np.empty((P, 4, LQB), np.float32)
    for m in range(4):
        mt[:, m, :] = (~cm[0, 0, 0:LQB, m * P:(m + 1) * P]).T
    mt = mt.astype(BF16NP)

    wq = np.asarray(wq, np.float32)
    wk = np.asarray(wk, np.float32)
    wv = np.asarray(wv, np.float32)
    wo = np.asarray(wo, np.float32)
    bq = np.asarray(bq, np.float32)
    bk = np.asarray(bk, np.float32)
    bv = np.asarray(bv, np.float32)

    in_maps = []
    for c in range(N_CORES):
        b = c // 4
        g = c % 4
        cols = slice(256 * g, 256 * (g + 1))

        xt = np.ascontiguousarray(
            x[b].T.reshape(KD, P, L).transpose(1, 0, 2))
        xhi, xlo = _split_fp8(xt)

        def pack_qk(w):
            # [D, 256] -> [p, g2, slot, pair, m], scaled
            a = (w[:, cols] * WS).reshape(4, 2, P, 2, P).transpose(2, 0, 1, 3, 4)
            hi, lo = _split_fp8(np.ascontiguousarray(a))
            return np.stack([hi, lo])

        def pack_v(w):
            a = (w[:, cols] * WS).reshape(4, 2, P, 2 * P).transpose(2, 0, 1, 3)
            hi, lo = _split_fp8(np.ascontiguousarray(a))
            return np.stack([hi, lo])

        in_maps.append({
            "xh": xhi,
            "xl": xlo,
            "wq8": pack_qk(wq),
            "wk8": pack_qk(wk),
            "wv8": pack_v(wv),
            "wo": np.ascontiguousarray(
                wo[cols, :].reshape(2, P, D)).astype(BF16NP),
            "bq": np.ascontiguousarray((bq[cols] * WS).reshape(2, P).T),
            "bk": np.ascontiguousarray((bk[cols] * WS).reshape(2, P).T),
            "bv": np.ascontiguousarray(bv[cols] * WS),
            "mask": mt,
        })
    return in_maps


def run(inputs, trace=False, mm_dt=None, iters=1, **kw):
    nc = _get_nc(mm_dt, iters)
    in_maps = _make_in_maps(
        inputs["x"], inputs["causal_mask"], inputs["wq"], inputs["bq"],
        inputs["wk"], inputs["bk"], inputs["wv"], inputs["bv"], inputs["wo"])
    res = run_bass_kernel_spmd(nc, in_maps, list(range(N_CORES)),
                               trace=trace, **kw)
    bo = np.asarray(inputs["bo"], np.float32)
    out = np.zeros((B, L, D), np.float32)
    for c in range(N_CORES):
        out[c // 4] += res.results[c]["out"].astype(np.float32)
    out += bo[None, None, :]
    return out, res


def kernel(**inputs):
    out, _ = run(inputs)
    return out


# revision 37
# speedup vs baseline: 1.0106x; 1.0023x over previous
"""Multi-head attention (B=2, L=2048, D=1024, H=16) on 8 TRN2 NeuronCores.

Sharding: core c handles batch b=c//4 and heads 4*(c%4) .. 4*(c%4)+3
(tensor-parallel over heads x data-parallel over batch). Each core computes a
partial [L, D] output (its heads' contribution through wo); the host sums the
4 partials per batch and adds bo.

Device-side design:
  - QKV projections run in fp8-e4m3 DoubleRow matmuls (0.5 cyc/row, two
    128-row contractions per instruction). Accuracy is preserved with a
    compensated split prepared on the host: x = xh + xl, w = wh + wl (wl/xl
    are the fp8 quantization residuals), and x@w is computed as
    xh@wh + xl@wh + xh@wl (the dropped xl@wl term is ~1e-3 relative).
    Weights are pre-scaled by 32 so their uniform(-1/32,1/32) range stays in
    fp8 normal range; biases are pre-scaled to match, and the V "ones"
    column carries the same scale so softmax normalization cancels it.
  - Q^T/K^T [2*Dh, L]; S^T [k, q] blocks of [128, 512] with exp on paired
    2-bank PSUM tiles; causal handled by skipping k-blocks above the
    diagonal, shrinking diagonal tiles to their unmasked column range, and
    bf16 multiplicative mask tiles for the intra-block triangles.
  - PV runs transposed-back: O [q, dh] via lhsT = P^T slice (full 128-wide
    stationary, N=65 moving V+ones) which halves PV row count vs the
    [dh, q] orientation and makes the softmax denominator per-partition
    (one reciprocal + tensor_scalar per q-subblock, no PE broadcast).
  - O [q, dh] -> O^T via XBAR DMA transposes (SBUF->SBUF, 14ns/tile).
  - Projections/attention/output are software-pipelined per 512-row q-chunk:
    within a chunk the two head-streams of a pair interleave at pair-tile
    granularity with PV one round behind S (hiding exp latency), and a
    filler queue spreads next-chunk projections and deferred output
    projections into the PE stream's exp-wait windows. Output-projection
    units of early chunks are deferred to the exp-bound late chunks, which
    would otherwise starve the PE. The partial output is written bf16 (the
    host accumulates cores in f32).
"""

from contextlib import ExitStack

import numpy as np
import ml_dtypes

import concourse.bass as bass
import concourse.mybir as mybir
import concourse.tile as tile
from concourse import bacc
from concourse.bass_utils import run_bass_kernel_spmd

B, L, D, H = 2, 2048, 1024, 16
DH = D // H          # 64
P = 128              # partitions
NPAIR = 2            # head pairs per core (4 heads)
LQB = 512            # q chunk
NLQ = L // LQB       # 4
NKB = L // P         # 16 k blocks
KD = D // P          # 8 contraction blocks over D
N_CORES = 8
WS = 32.0            # host-side weight scale (fp8 range)
SC = (1.0 / np.sqrt(DH)) / (WS * WS)   # exp scale: undo WS^2 in scores

F32 = mybir.dt.float32
BF16 = mybir.dt.bfloat16
FP8 = mybir.dt.float8e4
AF = mybir.ActivationFunctionType
DR = mybir.MatmulPerfMode.DoubleRow
E4M3 = ml_dtypes.float8_e4m3
BF16NP = ml_dtypes.bfloat16


def build_module(iters=1, dbg=False):
    nc = bacc.Bacc("TRN2", target_bir_lowering=False, debug=False,
                   num_devices=N_CORES)
    if dbg:
        dbg_qt = nc.dram_tensor("dbg_qt", [P, L], F32, kind="ExternalOutput").ap()
        dbg_kt = nc.dram_tensor("dbg_kt", [P, L], F32, kind="ExternalOutput").ap()
        dbg_vx = nc.dram_tensor("dbg_vx", [P, NKB, 2, DH + 1], F32,
                                kind="ExternalOutput").ap()
        dbg_osl = nc.dram_tensor("dbg_osl", [P, NKB, P], F32,
                                 kind="ExternalOutput").ap()
        dbg_otT = nc.dram_tensor("dbg_otT", [P, NKB, P], F32,
                                 kind="ExternalOutput").ap()

    xh_d = nc.dram_tensor("xh", [P, KD, L], FP8, kind="ExternalInput").ap()
    xl_d = nc.dram_tensor("xl", [P, KD, L], FP8, kind="ExternalInput").ap()
    # [hl, p, g2, slot, pair, m]
    wq_d = nc.dram_tensor("wq8", [2, P, 4, 2, 2, P], FP8, kind="ExternalInput").ap()
    wk_d = nc.dram_tensor("wk8", [2, P, 4, 2, 2, P], FP8, kind="ExternalInput").ap()
    # [hl, p, g2, slot, ch]
    wv_d = nc.dram_tensor("wv8", [2, P, 4, 2, 2 * P], FP8, kind="ExternalInput").ap()
    wo_d = nc.dram_tensor("wo", [2, P, D], BF16, kind="ExternalInput").ap()
    bq_d = nc.dram_tensor("bq", [P, 2], F32, kind="ExternalInput").ap()
    bk_d = nc.dram_tensor("bk", [P, 2], F32, kind="ExternalInput").ap()
    bv_d = nc.dram_tensor("bv", [2 * P], F32, kind="ExternalInput").ap()
    mask_d = nc.dram_tensor("mask", [P, 4, LQB], BF16, kind="ExternalInput").ap()
    out = nc.dram_tensor("out", [L, D], BF16, kind="ExternalOutput").ap()

    with tile.TileContext(nc) as tc, ExitStack() as ctx:
        ctx.enter_context(
            nc.allow_low_precision(reason="fp8/bf16 matmul data path"))
        consts = ctx.enter_context(tc.tile_pool(name="consts", bufs=1))
        pers = ctx.enter_context(tc.tile_pool(name="pers", bufs=1))
        work = ctx.enter_context(tc.tile_pool(name="work", bufs=1))
        ps = ctx.enter_context(tc.tile_pool(name="ps", bufs=1, space="PSUM"))

        # ---- const tiles ---------------------------------------------------
        xh_sb = consts.tile([P, KD, L], FP8, tag="xh")
        xl_sb = consts.tile([P, KD, L], FP8, tag="xl")
        wq_sb = consts.tile([P, 2, 4, 2, 2, P], FP8, tag="wq")
        wk_sb = consts.tile([P, 2, 4, 2, 2, P], FP8, tag="wk")
        wv_sb = consts.tile([P, 2, 4, 2, 2 * P], FP8, tag="wv")
        wo_sb = consts.tile([P, 2, D], BF16, tag="wo")
        bq_sb = consts.tile([P, 2], F32, tag="bq")
        bk_sb = consts.tile([P, 2], F32, tag="bk")
        bv_bc = consts.tile([P, 2 * P], F32, tag="bv")
        mask_sb = consts.tile([P, 4, LQB], BF16, tag="mask")

        # DMA order matters: weights for chunk-0 QK first, then x chunk
        # slabs interleaved with the tensors each chunk unlocks.
        def load_x_chunk(ci, hl):
            sl = slice(ci * LQB, (ci + 1) * LQB)
            src = (xh_d, xl_d)[hl]
            dst = (xh_sb, xl_sb)[hl]
            nc.sync.dma_start(out=dst[:, :, sl], in_=src[:, :, sl])

        nc.sync.dma_start(out=wq_sb[:, 0], in_=wq_d[0])
        load_x_chunk(0, 0)
        nc.sync.dma_start(out=wk_sb[:, 0], in_=wk_d[0])
        nc.sync.dma_start(out=bq_sb[:], in_=bq_d)
        nc.sync.dma_start(out=bk_sb[:], in_=bk_d)
        load_x_chunk(0, 1)
        nc.sync.dma_start(out=wq_sb[:, 1], in_=wq_d[1])
        nc.sync.dma_start(out=wk_sb[:, 1], in_=wk_d[1])
        nc.sync.dma_start(out=wv_sb[:, 0], in_=wv_d[0])
        nc.sync.dma_start(out=wv_sb[:, 1], in_=wv_d[1])
        bv_b = bass.AP(tensor=bv_d.tensor, offset=bv_d.offset,
                       ap=[[0, P]] + list(bv_d.ap))
        nc.gpsimd.dma_start(out=bv_bc[:], in_=bv_b)
        nc.sync.dma_start(out=mask_sb[:], in_=mask_d)
        load_x_chunk(1, 0)
        load_x_chunk(1, 1)
        for pair in range(2):
            nc.sync.dma_start(out=wo_sb[:, pair], in_=wo_d[pair])
        for ci in range(2, NLQ):
            load_x_chunk(ci, 0)
            load_x_chunk(ci, 1)

        # ---- persistent work tiles ----------------------------------------
        qt_t = [pers.tile([P, L], BF16, tag=f"qt{p}", name=f"qt{p}") for p in range(2)]
        kt_t = [pers.tile([P, L], BF16, tag=f"kt{p}", name=f"kt{p}") for p in range(2)]
        vx_t = [pers.tile([P, NKB, 2, DH + 1], BF16, tag=f"vx{p}", name=f"vx{p}")
                for p in range(2)]
        osl_t = [pers.tile([P, NKB, P], BF16, tag=f"osl{p}", name=f"osl{p}") for p in range(2)]
        otT_t = [pers.tile([P, NKB, P], BF16, tag=f"otT{p}", name=f"otT{p}") for p in range(2)]

        for _it in range(iters):
            for pair in range(2):
                nc.gpsimd.memset(vx_t[pair][:], WS)

            # (w-term, x-term) for the compensated product
            TERMS = ((0, xh_sb), (1, xh_sb), (0, xl_sb))

            def qk_proj(pair, which, ci):
                w_sb, b_sb, dst = (
                    (wq_sb, bq_sb, qt_t[pair]) if which == 0
                    else (wk_sb, bk_sb, kt_t[pair]))
                sl = slice(ci * LQB, (ci + 1) * LQB)
                acc = ps.tile([P, LQB], F32, tag="acc", bufs=2)
                n = 0
                for wt, x_sb in TERMS:
                    for g2 in range(4):
                        nc.tensor.matmul(
                            acc[:],
                            w_sb[:, wt, g2, :, pair, :],
                            x_sb[:, 2 * g2:2 * g2 + 2, sl],
                            start=(n == 0), stop=(n == 11), perf_mode=DR)
                        n += 1
                nc.vector.tensor_scalar_add(dst[:, sl], acc[:],
                                            b_sb[:, pair:pair + 1])

            def v_proj(j):
                acc = ps.tile([P, 2 * P], F32, tag="acc", bufs=2)
                jsl = slice(j * P, (j + 1) * P)
                n = 0
                for wt, x_sb in TERMS:
                    for g2 in range(4):
                        nc.tensor.matmul(
                            acc[:],
                            x_sb[:, 2 * g2:2 * g2 + 2, jsl],
                            wv_sb[:, wt, g2, :, :],
                            start=(n == 0), stop=(n == 11), perf_mode=DR)
                        n += 1
                for pair in range(2):
                    for h in range(2):
                        c0 = pair * P + h * DH
                        nc.vector.tensor_add(
                            vx_t[pair][:, j, h, 0:DH],
                            acc[:, c0:c0 + DH], bv_bc[:, c0:c0 + DH])

            def s_group(pair, h, ci, jp):
                """S matmuls + exp (+ masks) for pair-tile jp; returns pt."""
                qt, kt = qt_t[pair], kt_t[pair]
                hp = h * DH
                s = ps.tile([P, 2, LQB], F32, tag="s", bufs=2)
                pt = work.tile([P, 2, LQB], BF16, tag="pt", bufs=8)
                poff = 0 if jp <= 2 * ci else 2 * P
                for jj in range(2):
                    j = 2 * jp + jj
                    off = poff if jp >= 2 * ci else 0
                    nc.tensor.matmul(
                        s[:, jj, off:LQB],
                        kt[hp:hp + DH, j * P:(j + 1) * P],
                        qt[hp:hp + DH, ci * LQB + off:(ci + 1) * LQB],
                        start=True, stop=True)
                if jp < 2 * ci:
                    nc.scalar.activation(pt[:], s[:], AF.Exp, scale=SC)
                else:
                    nc.scalar.activation(pt[:, :, poff:LQB],
                                         s[:, :, poff:LQB], AF.Exp, scale=SC)
                    for jj in range(2):
                        m = 2 * jp + jj - 4 * ci
                        if m >= 0:
                            nc.vector.tensor_mul(
                                pt[:, jj, poff:LQB], pt[:, jj, poff:LQB],
                                mask_sb[:, m, poff:LQB])
                return pt

            def pv_group(pair, h, ci, jp, pt, ot):
                # ot is one PSUM bank: hardware start zeroes the whole bank,
                # so the (head, chunk) group has exactly one start (first
                # matmul) and one stop (last matmul).
                vx = vx_t[pair]
                for jj in range(2):
                    j = 2 * jp + jj
                    m = j - 4 * ci
                    for sb in range(max(0, m), 4):
                        nc.tensor.matmul(
                            ot[:, sb, 0:DH + 1],
                            pt[:, jj, sb * P:(sb + 1) * P],
                            vx[:, j, h, :],
                            start=(j == 0 and sb == 0),
                            stop=(j == 4 * ci + 3 and sb == 3),
                            skip_group_check=True)

            def normalize(pair, h, ci, ot):
                hp = h * DH
                rec = work.tile([P, 4], F32, tag="rec", bufs=4)
                nc.vector.reciprocal(rec[:], ot[:, :, DH])
                for sb in range(4):
                    nc.vector.tensor_scalar_mul(
                        osl_t[pair][:, 4 * ci + sb, hp:hp + DH],
                        ot[:, sb, 0:DH], rec[:, sb:sb + 1])

            def outproj_half(lb, half, osb):
                acc = ps.tile([P, LQB], F32, tag="acc", bufs=2)
                for pair in range(2):
                    nc.tensor.matmul(
                        acc[:],
                        otT_t[pair][:, lb, :],
                        wo_sb[:, pair, half * LQB:(half + 1) * LQB],
                        start=(pair == 0), stop=(pair == 1))
                nc.vector.tensor_copy(osb[:, half * LQB:(half + 1) * LQB],
                                      acc[:])

            def outproj_units(ci):
                units = []
                for lb in range(4 * ci, 4 * ci + 4):
                    osb = work.tile([P, D], BF16, tag="osb", bufs=4,
                                    name=f"osb{lb}")
                    for half in range(2):
                        def unit(l=lb, o=osb, hf=half):
                            outproj_half(l, hf, o)
                            nc.sync.dma_start(
                                out=out[l * P:(l + 1) * P,
                                        hf * LQB:(hf + 1) * LQB],
                                in_=o[:, hf * LQB:(hf + 1) * LQB])
                        units.append(unit)
                return units

            # prologue: chunk 0 projections for pair 0 only; pair 1 comes
            # through the filler queue during pair-0 attention. Q and K are
            # staged hi-terms-first so K's hi matmuls overlap the xl DMA.
            pro_accs = []
            for which in range(2):
                w_sb = (wq_sb, wk_sb)[which]
                acc = ps.tile([P, LQB], F32, tag="acc", bufs=2,
                              name=f"proacc{which}")
                for g2 in range(4):
                    nc.tensor.matmul(
                        acc[:], w_sb[:, 0, g2, :, 0, :],
                        xh_sb[:, 2 * g2:2 * g2 + 2, 0:LQB],
                        start=(g2 == 0), stop=False, perf_mode=DR)
                pro_accs.append(acc)
            for which in range(2):
                w_sb, b_sb, dst = ((wq_sb, bq_sb, qt_t[0]),
                                   (wk_sb, bk_sb, kt_t[0]))[which]
                acc = pro_accs[which]
                n = 0
                for wt, x_sb in ((1, xh_sb), (0, xl_sb)):
                    for g2 in range(4):
                        nc.tensor.matmul(
                            acc[:], w_sb[:, wt, g2, :, 0, :],
                            x_sb[:, 2 * g2:2 * g2 + 2, 0:LQB],
                            start=False, stop=(n == 7), perf_mode=DR)
                        n += 1
                nc.vector.tensor_scalar_add(dst[:, 0:LQB], acc[:],
                                            b_sb[:, 0:1])
            for j in range(4):
                v_proj(j)

            # steady state: per chunk, the two head-streams of each pair are
            # interleaved at pair-tile granularity with PV pipelined one
            # round behind S, and a filler queue (next-chunk projections,
            # prev-chunk output projection) feeds the PE stream's exp-wait
            # windows.
            fillers = [lambda: qk_proj(1, 0, 0), lambda: qk_proj(1, 1, 0)]
            deferred = []
            quota = [0.0]

            def drain(slots_left, rate=1.0):
                # spread remaining fillers over remaining drain slots; rate>1
                # front-loads (for units with a chunk-boundary deadline)
                quota[0] += rate * len(fillers) / max(1.0, slots_left)
                while quota[0] >= 1.0 and fillers:
                    quota[0] -= 1.0
                    fillers.pop(0)()

            for ci in range(NLQ):
                nxt = ci + 1
                if nxt < NLQ:
                    for pair in range(2):
                        fillers.append(lambda p=pair: qk_proj(p, 0, nxt))
                        fillers.append(lambda p=pair: qk_proj(p, 1, nxt))
                    for j in range(4 * nxt, 4 * nxt + 4):
                        fillers.append(lambda jj=j: v_proj(jj))
                if ci == NLQ - 1:
                    # late chunks are exp-bound and filler-starved: feed them
                    # the deferred output-projection units
                    fillers.extend(deferred)
                    deferred = []
                nrounds = 2 * ci + 2
                slots = 4 * nrounds
                for pair in range(2):
                    ot_h = [ps.tile([P, 4, P], F32, tag="ot", name=f"ot{h}", bufs=2)
                            for h in range(2)]
                    pt_prev = [None, None]
                    for jp in range(nrounds):
                        for h in ((0, 1) if (jp + pair) % 2 == 0 else (1, 0)):
                            pt = s_group(pair, h, ci, jp)
                            drain(slots, 1.0)
                            slots -= 1
                            if pt_prev[h] is not None:
                                pv_group(pair, h, ci, jp - 1, pt_prev[h],
                                         ot_h[h])
                            pt_prev[h] = pt
                    for h in range(2):
                        pv_group(pair, h, ci, nrounds - 1, pt_prev[h], ot_h[h])
                        normalize(pair, h, ci, ot_h[h])
                    for qb in range(4 * ci, 4 * ci + 2):
                        nc.sync.dma_start(out=otT_t[pair][:, qb, :],
                                          in_=osl_t[pair][:, qb, :],
                                          transpose=True)
                    nc.sync.dma_start(
                        out=otT_t[pair][:, 4 * ci + 2:4 * ci + 4, :],
                        in_=osl_t[pair][:, 4 * ci + 2:4 * ci + 4, :],
                        transpose=True)
                while fillers:
                    fillers.pop(0)()
                if ci >= 2:
                    fillers.extend(outproj_units(ci))
                else:
                    deferred.extend(outproj_units(ci))
            while fillers:
                fillers.pop(0)()

        if dbg:
            for name_, src, dst in (("qt", qt_t[0], dbg_qt),
                                    ("kt", kt_t[0], dbg_kt),
                                    ("vx", vx_t[0], dbg_vx),
                                    ("osl", osl_t[0], dbg_osl),
                                    ("otT", otT_t[0], dbg_otT)):
                tmp = work.tile(list(src.shape), F32, tag=f"dbg{name_}",
                                name=f"dbg{name_}")
                nc.vector.tensor_copy(tmp[:], src[:])
                nc.sync.dma_start(out=dst, in_=tmp[:])

    nc.compile()
    return nc


_CACHE = {}


def _get_nc(mm_dt=None, iters=1):
    key = iters
    if key not in _CACHE:
        _CACHE[key] = build_module(iters)
    return _CACHE[key]


def _split_fp8(a):
    hi = a.astype(E4M3)
    lo = (a - hi.astype(np.float32)).astype(E4M3)
    return hi, lo


def _make_in_maps(x, causal_mask, wq, bq, wk, bk, wv, bv, wo):
    x = np.asarray(x, np.float32)
    cm = np.asarray(causal_mask)
    # mask tile m (for k-block j = 4i+m): keep[p, c] = (c >= 128m + p)
    mt = np.empty((P, 4, LQB), np.float32)
    for m in range(4):
        mt[:, m, :] = (~cm[0, 0, 0:LQB, m * P:(m + 1) * P]).T
    mt = mt.astype(BF16NP)

    wq = np.asarray(wq, np.float32)
    wk = np.asarray(wk, np.float32)
    wv = np.asarray(wv, np.float32)
    wo = np.asarray(wo, np.float32)
    bq = np.asarray(bq, np.float32)
    bk = np.asarray(bk, np.float32)
    bv = np.asarray(bv, np.float32)

    in_maps = []
    for c in range(N_CORES):
        b = c // 4
        g = c % 4
        cols = slice(256 * g, 256 * (g + 1))

        xt = np.ascontiguousarray(
            x[b].T.reshape(KD, P, L).transpose(1, 0, 2))
        xhi, xlo = _split_fp8(xt)

        def pack_qk(w):
            # [D, 256] -> [p, g2, slot, pair, m], scaled
            a = (w[:, cols] * WS).reshape(4, 2, P, 2, P).transpose(2, 0, 1, 3, 4)
            hi, lo = _split_fp8(np.ascontiguousarray(a))
            return np.stack([hi, lo])

        def pack_v(w):
            a = (w[:, cols] * WS).reshape(4, 2, P, 2 * P).transpose(2, 0, 1, 3)
            hi, lo = _split_fp8(np.ascontiguousarray(a))
            return np.stack([hi, lo])

        in_maps.append({
            "xh": xhi,
            "xl": xlo,
            "wq8": pack_qk(wq),
            "wk8": pack_qk(wk),
            "wv8": pack_v(wv),
            "wo": np.ascontiguousarray(
                wo[cols, :].reshape(2, P, D)).astype(BF16NP),
            "bq": np.ascontiguousarray((bq[cols] * WS).reshape(2, P).T),
            "bk": np.ascontiguousarray((bk[cols] * WS).reshape(2, P).T),
            "bv": np.ascontiguousarray(bv[cols] * WS),
            "mask": mt,
        })
    return in_maps


def run(inputs, trace=False, mm_dt=None, iters=1, **kw):
    nc = _get_nc(mm_dt, iters)
    in_maps = _make_in_maps(
        inputs["x"], inputs["causal_mask"], inputs["wq"], inputs["bq"],
        inputs["wk"], inputs["bk"], inputs["wv"], inputs["bv"], inputs["wo"])
    res = run_bass_kernel_spmd(nc, in_maps, list(range(N_CORES)),
                               trace=trace, **kw)
    bo = np.asarray(inputs["bo"], np.float32)
    out = np.zeros((B, L, D), np.float32)
    for c in range(N_CORES):
        out[c // 4] += res.results[c]["out"].astype(np.float32)
    out += bo[None, None, :]
    return out, res


def kernel(**inputs):
    out, _ = run(inputs)
    return out


# revision 38
# speedup vs baseline: 1.0148x; 1.0041x over previous
"""Multi-head attention (B=2, L=2048, D=1024, H=16) on 8 TRN2 NeuronCores.

Sharding: core c handles batch b=c//4 and heads 4*(c%4) .. 4*(c%4)+3
(tensor-parallel over heads x data-parallel over batch). Each core computes a
partial [L, D] output (its heads' contribution through wo); the host sums the
4 partials per batch and adds bo.

Device-side design:
  - QKV projections run in fp8-e4m3 DoubleRow matmuls (0.5 cyc/row, two
    128-row contractions per instruction). Accuracy is preserved with a
    compensated split prepared on the host: x = xh + xl, w = wh + wl (wl/xl
    are the fp8 quantization residuals), and x@w is computed as
    xh@wh + xl@wh + xh@wl (the dropped xl@wl term is ~1e-3 relative).
    Weights are pre-scaled by 32 so their uniform(-1/32,1/32) range stays in
    fp8 normal range; biases are pre-scaled to match, and the V "ones"
    column carries the same scale so softmax normalization cancels it.
  - Q^T/K^T [2*Dh, L]; S^T [k, q] blocks of [128, 512] with exp on paired
    2-bank PSUM tiles; causal handled by skipping k-blocks above the
    diagonal, shrinking diagonal tiles to their unmasked column range, and
    bf16 multiplicative mask tiles for the intra-block triangles.
  - PV runs transposed-back: O [q, dh] via lhsT = P^T slice (full 128-wide
    stationary, N=65 moving V+ones) which halves PV row count vs the
    [dh, q] orientation and makes the softmax denominator per-partition
    (one reciprocal + tensor_scalar per q-subblock, no PE broadcast).
  - O [q, dh] -> O^T via XBAR DMA transposes (SBUF->SBUF, 14ns/tile).
  - Projections/attention/output are software-pipelined per 512-row q-chunk:
    within a chunk the two head-streams of a pair interleave at pair-tile
    granularity with PV one round behind S (hiding exp latency), and a
    filler queue spreads next-chunk projections and deferred output
    projections into the PE stream's exp-wait windows. Output-projection
    units of early chunks are deferred to the exp-bound late chunks, which
    would otherwise starve the PE. The partial output is written bf16 (the
    host accumulates cores in f32).
"""

from contextlib import ExitStack

import numpy as np
import ml_dtypes

import concourse.bass as bass
import concourse.mybir as mybir
import concourse.tile as tile
from concourse import bacc
from concourse.bass_utils import run_bass_kernel_spmd

B, L, D, H = 2, 2048, 1024, 16
DH = D // H          # 64
P = 128              # partitions
NPAIR = 2            # head pairs per core (4 heads)
LQB = 512            # q chunk
NLQ = L // LQB       # 4
NKB = L // P         # 16 k blocks
KD = D // P          # 8 contraction blocks over D
N_CORES = 8
WS = 32.0            # host-side weight scale (fp8 range)
SC = (1.0 / np.sqrt(DH)) / (WS * WS)   # exp scale: undo WS^2 in scores

F32 = mybir.dt.float32
BF16 = mybir.dt.bfloat16
FP8 = mybir.dt.float8e4
AF = mybir.ActivationFunctionType
DR = mybir.MatmulPerfMode.DoubleRow
E4M3 = ml_dtypes.float8_e4m3
BF16NP = ml_dtypes.bfloat16


def build_module(iters=1, dbg=False):
    nc = bacc.Bacc("TRN2", target_bir_lowering=False, debug=False,
                   num_devices=N_CORES)
    if dbg:
        dbg_qt = nc.dram_tensor("dbg_qt", [P, L], F32, kind="ExternalOutput").ap()
        dbg_kt = nc.dram_tensor("dbg_kt", [P, L], F32, kind="ExternalOutput").ap()
        dbg_vx = nc.dram_tensor("dbg_vx", [P, NKB, 2, DH + 1], F32,
                                kind="ExternalOutput").ap()
        dbg_osl = nc.dram_tensor("dbg_osl", [P, NKB, P], F32,
                                 kind="ExternalOutput").ap()
        dbg_otT = nc.dram_tensor("dbg_otT", [P, NKB, P], F32,
                                 kind="ExternalOutput").ap()

    xh_d = nc.dram_tensor("xh", [P, KD, L], FP8, kind="ExternalInput").ap()
    xl_d = nc.dram_tensor("xl", [P, KD, L], FP8, kind="ExternalInput").ap()
    # [hl, p, g2, slot, pair, m]
    wq_d = nc.dram_tensor("wq8", [2, P, 4, 2, 2, P], FP8, kind="ExternalInput").ap()
    wk_d = nc.dram_tensor("wk8", [2, P, 4, 2, 2, P], FP8, kind="ExternalInput").ap()
    # [hl, p, g2, slot, ch]
    wv_d = nc.dram_tensor("wv8", [2, P, 4, 2, 2 * P], FP8, kind="ExternalInput").ap()
    wo_d = nc.dram_tensor("wo", [2, P, D], BF16, kind="ExternalInput").ap()
    bq_d = nc.dram_tensor("bq", [P, 2], F32, kind="ExternalInput").ap()
    bk_d = nc.dram_tensor("bk", [P, 2], F32, kind="ExternalInput").ap()
    bv_d = nc.dram_tensor("bv", [2 * P], F32, kind="ExternalInput").ap()
    mask_d = nc.dram_tensor("mask", [P, 4, LQB], BF16, kind="ExternalInput").ap()
    out = nc.dram_tensor("out", [L, D], BF16, kind="ExternalOutput").ap()

    with tile.TileContext(nc) as tc, ExitStack() as ctx:
        ctx.enter_context(
            nc.allow_low_precision(reason="fp8/bf16 matmul data path"))
        consts = ctx.enter_context(tc.tile_pool(name="consts", bufs=1))
        pers = ctx.enter_context(tc.tile_pool(name="pers", bufs=1))
        work = ctx.enter_context(tc.tile_pool(name="work", bufs=1))
        ps = ctx.enter_context(tc.tile_pool(name="ps", bufs=1, space="PSUM"))

        # ---- const tiles ---------------------------------------------------
        xh_sb = consts.tile([P, KD, L], FP8, tag="xh")
        xl_sb = consts.tile([P, KD, L], FP8, tag="xl")
        wq_sb = consts.tile([P, 2, 4, 2, 2, P], FP8, tag="wq")
        wk_sb = consts.tile([P, 2, 4, 2, 2, P], FP8, tag="wk")
        wv_sb = consts.tile([P, 2, 4, 2, 2 * P], FP8, tag="wv")
        wo_sb = consts.tile([P, 2, D], BF16, tag="wo")
        bq_sb = consts.tile([P, 2], F32, tag="bq")
        bk_sb = consts.tile([P, 2], F32, tag="bk")
        bv_bc = consts.tile([P, 2 * P], F32, tag="bv")
        mask_sb = consts.tile([P, 4, LQB], BF16, tag="mask")

        # DMA order matters: weights for chunk-0 QK first, then x chunk
        # slabs interleaved with the tensors each chunk unlocks.
        def load_x_chunk(ci, hl):
            sl = slice(ci * LQB, (ci + 1) * LQB)
            src = (xh_d, xl_d)[hl]
            dst = (xh_sb, xl_sb)[hl]
            nc.sync.dma_start(out=dst[:, :, sl], in_=src[:, :, sl])

        nc.sync.dma_start(out=wq_sb[:, 0], in_=wq_d[0])
        load_x_chunk(0, 0)
        nc.sync.dma_start(out=wk_sb[:, 0], in_=wk_d[0])
        nc.sync.dma_start(out=bq_sb[:], in_=bq_d)
        nc.sync.dma_start(out=bk_sb[:], in_=bk_d)
        load_x_chunk(0, 1)
        nc.sync.dma_start(out=wq_sb[:, 1], in_=wq_d[1])
        nc.sync.dma_start(out=wk_sb[:, 1], in_=wk_d[1])
        nc.sync.dma_start(out=wv_sb[:, 0], in_=wv_d[0])
        nc.sync.dma_start(out=wv_sb[:, 1], in_=wv_d[1])
        bv_b = bass.AP(tensor=bv_d.tensor, offset=bv_d.offset,
                       ap=[[0, P]] + list(bv_d.ap))
        nc.gpsimd.dma_start(out=bv_bc[:], in_=bv_b)
        nc.sync.dma_start(out=mask_sb[:], in_=mask_d)
        load_x_chunk(1, 0)
        load_x_chunk(1, 1)
        for pair in range(2):
            nc.sync.dma_start(out=wo_sb[:, pair], in_=wo_d[pair])
        for ci in range(2, NLQ):
            load_x_chunk(ci, 0)
            load_x_chunk(ci, 1)

        # ---- persistent work tiles ----------------------------------------
        qt_t = [pers.tile([P, L], BF16, tag=f"qt{p}", name=f"qt{p}") for p in range(2)]
        kt_t = [pers.tile([P, L], BF16, tag=f"kt{p}", name=f"kt{p}") for p in range(2)]
        vx_t = [pers.tile([P, NKB, 2, DH + 1], BF16, tag=f"vx{p}", name=f"vx{p}")
                for p in range(2)]
        osl_t = [pers.tile([P, NKB, P], BF16, tag=f"osl{p}", name=f"osl{p}") for p in range(2)]
        otT_t = [pers.tile([P, NKB, P], BF16, tag=f"otT{p}", name=f"otT{p}") for p in range(2)]

        for _it in range(iters):
            for pair in range(2):
                nc.gpsimd.memset(vx_t[pair][:], WS)

            # (w-term, x-term) for the compensated product
            TERMS = ((0, xh_sb), (1, xh_sb), (0, xl_sb))

            def qk_proj(pair, which, ci):
                w_sb, b_sb, dst = (
                    (wq_sb, bq_sb, qt_t[pair]) if which == 0
                    else (wk_sb, bk_sb, kt_t[pair]))
                sl = slice(ci * LQB, (ci + 1) * LQB)
                acc = ps.tile([P, LQB], F32, tag="acc", bufs=2)
                n = 0
                for wt, x_sb in TERMS:
                    for g2 in range(4):
                        nc.tensor.matmul(
                            acc[:],
                            w_sb[:, wt, g2, :, pair, :],
                            x_sb[:, 2 * g2:2 * g2 + 2, sl],
                            start=(n == 0), stop=(n == 11), perf_mode=DR)
                        n += 1
                nc.vector.tensor_scalar_add(dst[:, sl], acc[:],
                                            b_sb[:, pair:pair + 1])

            def v_proj(j):
                acc = ps.tile([P, 2 * P], F32, tag="acc", bufs=2)
                jsl = slice(j * P, (j + 1) * P)
                n = 0
                for wt, x_sb in TERMS:
                    for g2 in range(4):
                        nc.tensor.matmul(
                            acc[:],
                            x_sb[:, 2 * g2:2 * g2 + 2, jsl],
                            wv_sb[:, wt, g2, :, :],
                            start=(n == 0), stop=(n == 11), perf_mode=DR)
                        n += 1
                for pair in range(2):
                    for h in range(2):
                        c0 = pair * P + h * DH
                        nc.vector.tensor_add(
                            vx_t[pair][:, j, h, 0:DH],
                            acc[:, c0:c0 + DH], bv_bc[:, c0:c0 + DH])

            def s_group(pair, h, ci, jp):
                """S matmuls + exp (+ masks) for pair-tile jp; returns pt."""
                qt, kt = qt_t[pair], kt_t[pair]
                hp = h * DH
                s = ps.tile([P, 2, LQB], F32, tag="s", bufs=2)
                pt = work.tile([P, 2, LQB], BF16, tag="pt", bufs=8)
                poff = 0 if jp <= 2 * ci else 2 * P
                for jj in range(2):
                    j = 2 * jp + jj
                    off = poff if jp >= 2 * ci else 0
                    nc.tensor.matmul(
                        s[:, jj, off:LQB],
                        kt[hp:hp + DH, j * P:(j + 1) * P],
                        qt[hp:hp + DH, ci * LQB + off:(ci + 1) * LQB],
                        start=True, stop=True)
                if jp < 2 * ci:
                    nc.scalar.activation(pt[:], s[:], AF.Exp, scale=SC)
                else:
                    nc.scalar.activation(pt[:, :, poff:LQB],
                                         s[:, :, poff:LQB], AF.Exp, scale=SC)
                    for jj in range(2):
                        m = 2 * jp + jj - 4 * ci
                        if m >= 0:
                            nc.vector.tensor_mul(
                                pt[:, jj, poff:LQB], pt[:, jj, poff:LQB],
                                mask_sb[:, m, poff:LQB])
                return pt

            def pv_group(pair, h, ci, jp, pt, ot):
                # ot is one PSUM bank: hardware start zeroes the whole bank,
                # so the (head, chunk) group has exactly one start (first
                # matmul) and one stop (last matmul).
                vx = vx_t[pair]
                for jj in range(2):
                    j = 2 * jp + jj
                    m = j - 4 * ci
                    for sb in range(max(0, m), 4):
                        nc.tensor.matmul(
                            ot[:, sb, 0:DH + 1],
                            pt[:, jj, sb * P:(sb + 1) * P],
                            vx[:, j, h, :],
                            start=(j == 0 and sb == 0),
                            stop=(j == 4 * ci + 3 and sb == 3),
                            skip_group_check=True)

            def normalize(pair, h, ci, ot):
                hp = h * DH
                rec = work.tile([P, 4], F32, tag="rec", bufs=4)
                nc.vector.reciprocal(rec[:], ot[:, :, DH])
                for sb in range(4):
                    nc.vector.tensor_scalar_mul(
                        osl_t[pair][:, 4 * ci + sb, hp:hp + DH],
                        ot[:, sb, 0:DH], rec[:, sb:sb + 1])

            def outproj_half(lb, half, osb):
                acc = ps.tile([P, LQB], F32, tag="acc", bufs=2)
                for pair in range(2):
                    nc.tensor.matmul(
                        acc[:],
                        otT_t[pair][:, lb, :],
                        wo_sb[:, pair, half * LQB:(half + 1) * LQB],
                        start=(pair == 0), stop=(pair == 1))
                nc.vector.tensor_copy(osb[:, half * LQB:(half + 1) * LQB],
                                      acc[:])

            def outproj_units(ci):
                units = []
                for lb in range(4 * ci, 4 * ci + 4):
                    osb = work.tile([P, D], BF16, tag="osb", bufs=4,
                                    name=f"osb{lb}")
                    for half in range(2):
                        def unit(l=lb, o=osb, hf=half):
                            outproj_half(l, hf, o)
                            nc.sync.dma_start(
                                out=out[l * P:(l + 1) * P,
                                        hf * LQB:(hf + 1) * LQB],
                                in_=o[:, hf * LQB:(hf + 1) * LQB])
                        units.append(unit)
                return units

            # prologue: chunk 0 projections for pair 0 only; pair 1 comes
            # through the filler queue during pair-0 attention. Q and K are
            # staged hi-terms-first so K's hi matmuls overlap the xl DMA.
            pro_accs = []
            for which in range(2):
                w_sb = (wq_sb, wk_sb)[which]
                acc = ps.tile([P, LQB], F32, tag="acc", bufs=2,
                              name=f"proacc{which}")
                for g2 in range(4):
                    nc.tensor.matmul(
                        acc[:], w_sb[:, 0, g2, :, 0, :],
                        xh_sb[:, 2 * g2:2 * g2 + 2, 0:LQB],
                        start=(g2 == 0), stop=False, perf_mode=DR)
                pro_accs.append(acc)
            for which in range(2):
                w_sb, b_sb, dst = ((wq_sb, bq_sb, qt_t[0]),
                                   (wk_sb, bk_sb, kt_t[0]))[which]
                acc = pro_accs[which]
                n = 0
                for wt, x_sb in ((1, xh_sb), (0, xl_sb)):
                    for g2 in range(4):
                        nc.tensor.matmul(
                            acc[:], w_sb[:, wt, g2, :, 0, :],
                            x_sb[:, 2 * g2:2 * g2 + 2, 0:LQB],
                            start=False, stop=(n == 7), perf_mode=DR)
                        n += 1
                nc.vector.tensor_scalar_add(dst[:, 0:LQB], acc[:],
                                            b_sb[:, 0:1])
            for j in range(4):
                v_proj(j)

            # steady state: per chunk, the two head-streams of each pair are
            # interleaved at pair-tile granularity with PV pipelined one
            # round behind S, and a filler queue (next-chunk projections,
            # prev-chunk output projection) feeds the PE stream's exp-wait
            # windows.
            fillers = [lambda: qk_proj(1, 0, 0), lambda: qk_proj(1, 1, 0)]
            deferred = []
            quota = [0.0]

            def drain(slots_left, rate=1.0):
                # spread remaining fillers over remaining drain slots; rate>1
                # front-loads (for units with a chunk-boundary deadline)
                quota[0] += rate * len(fillers) / max(1.0, slots_left)
                while quota[0] >= 1.0 and fillers:
                    quota[0] -= 1.0
                    fillers.pop(0)()

            for ci in range(NLQ):
                nxt = ci + 1
                if nxt < NLQ:
                    for pair in range(2):
                        fillers.append(lambda p=pair: qk_proj(p, 0, nxt))
                        fillers.append(lambda p=pair: qk_proj(p, 1, nxt))
                    for j in range(4 * nxt, 4 * nxt + 4):
                        fillers.append(lambda jj=j: v_proj(jj))
                if ci == NLQ - 1:
                    # late chunks are exp-bound and filler-starved: feed them
                    # the deferred output-projection units
                    fillers.extend(deferred)
                    deferred = []
                nrounds = 2 * ci + 2
                slots = 12 * nrounds
                for pair in range(2):
                    ot_h = [ps.tile([P, 4, P], F32, tag="ot", name=f"ot{h}", bufs=2)
                            for h in range(2)]
                    pt_prev = [None, None]
                    for jp in range(nrounds):
                        for h in ((0, 1) if (jp + pair) % 2 == 0 else (1, 0)):
                            drain(slots, 1.0)
                            slots -= 1
                            pt = s_group(pair, h, ci, jp)
                            drain(slots, 1.0)
                            slots -= 1
                            if pt_prev[h] is not None:
                                pv_group(pair, h, ci, jp - 1, pt_prev[h],
                                         ot_h[h])
                            drain(slots, 1.0)
                            slots -= 1
                            pt_prev[h] = pt
                    for h in range(2):
                        pv_group(pair, h, ci, nrounds - 1, pt_prev[h], ot_h[h])
                        normalize(pair, h, ci, ot_h[h])
                    for qb in range(4 * ci, 4 * ci + 2):
                        nc.sync.dma_start(out=otT_t[pair][:, qb, :],
                                          in_=osl_t[pair][:, qb, :],
                                          transpose=True)
                    nc.sync.dma_start(
                        out=otT_t[pair][:, 4 * ci + 2:4 * ci + 4, :],
                        in_=osl_t[pair][:, 4 * ci + 2:4 * ci + 4, :],
                        transpose=True)
                while fillers:
                    fillers.pop(0)()
                if ci >= 2:
                    fillers.extend(outproj_units(ci))
                else:
                    deferred.extend(outproj_units(ci))
            while fillers:
                fillers.pop(0)()

        if dbg:
            for name_, src, dst in (("qt", qt_t[0], dbg_qt),
                                    ("kt", kt_t[0], dbg_kt),
                                    ("vx", vx_t[0], dbg_vx),
                                    ("osl", osl_t[0], dbg_osl),
                                    ("otT", otT_t[0], dbg_otT)):
                tmp = work.tile(list(src.shape), F32, tag=f"dbg{name_}",
                                name=f"dbg{name_}")
                nc.vector.tensor_copy(tmp[:], src[:])
                nc.sync.dma_start(out=dst, in_=tmp[:])

    nc.compile()
    return nc


_CACHE = {}


def _get_nc(mm_dt=None, iters=1):
    key = iters
    if key not in _CACHE:
        _CACHE[key] = build_module(iters)
    return _CACHE[key]


def _split_fp8(a):
    hi = a.astype(E4M3)
    lo = (a - hi.astype(np.float32)).astype(E4M3)
    return hi, lo


def _make_in_maps(x, causal_mask, wq, bq, wk, bk, wv, bv, wo):
    x = np.asarray(x, np.float32)
    cm = np.asarray(causal_mask)
    # mask tile m (for k-block j = 4i+m): keep[p, c] = (c >= 128m + p)
    mt = np.empty((P, 4, LQB), np.float32)
    for m in range(4):
        mt[:, m, :] = (~cm[0, 0, 0:LQB, m * P:(m + 1) * P]).T
    mt = mt.astype(BF16NP)

    wq = np.asarray(wq, np.float32)
    wk = np.asarray(wk, np.float32)
    wv = np.asarray(wv, np.float32)
    wo = np.asarray(wo, np.float32)
    bq = np.asarray(bq, np.float32)
    bk = np.asarray(bk, np.float32)
    bv = np.asarray(bv, np.float32)

    in_maps = []
    for c in range(N_CORES):
        b = c // 4
        g = c % 4
        cols = slice(256 * g, 256 * (g + 1))

        xt = np.ascontiguousarray(
            x[b].T.reshape(KD, P, L).transpose(1, 0, 2))
        xhi, xlo = _split_fp8(xt)

        def pack_qk(w):
            # [D, 256] -> [p, g2, slot, pair, m], scaled
            a = (w[:, cols] * WS).reshape(4, 2, P, 2, P).transpose(2, 0, 1, 3, 4)
            hi, lo = _split_fp8(np.ascontiguousarray(a))
            return np.stack([hi, lo])

        def pack_v(w):
            a = (w[:, cols] * WS).reshape(4, 2, P, 2 * P).transpose(2, 0, 1, 3)
            hi, lo = _split_fp8(np.ascontiguousarray(a))
            return np.stack([hi, lo])

        in_maps.append({
            "xh": xhi,
            "xl": xlo,
            "wq8": pack_qk(wq),
            "wk8": pack_qk(wk),
            "wv8": pack_v(wv),
            "wo": np.ascontiguousarray(
                wo[cols, :].reshape(2, P, D)).astype(BF16NP),
            "bq": np.ascontiguousarray((bq[cols] * WS).reshape(2, P).T),
            "bk": np.ascontiguousarray((bk[cols] * WS).reshape(2, P).T),
            "bv": np.ascontiguousarray(bv[cols] * WS),
            "mask": mt,
        })
    return in_maps


def run(inputs, trace=False, mm_dt=None, iters=1, **kw):
    nc = _get_nc(mm_dt, iters)
    in_maps = _make_in_maps(
        inputs["x"], inputs["causal_mask"], inputs["wq"], inputs["bq"],
        inputs["wk"], inputs["bk"], inputs["wv"], inputs["bv"], inputs["wo"])
    res = run_bass_kernel_spmd(nc, in_maps, list(range(N_CORES)),
                               trace=trace, **kw)
    bo = np.asarray(inputs["bo"], np.float32)
    out = np.zeros((B, L, D), np.float32)
    for c in range(N_CORES):
        out[c // 4] += res.results[c]["out"].astype(np.float32)
    out += bo[None, None, :]
    return out, res


def kernel(**inputs):
    out, _ = run(inputs)
    return out
